# revision 10
# baseline (speedup 1.0000x reference)
"""Trainium2 Bass kernel for nn_AttentionHead (B=4, S=2048, D_IN=D_OUT=1024).

Sharding: 8 cores; core c handles batch b=c//2, parity h=c%2.  Queries are
64-interleaved: core h takes queries [128*qt + 64*h, 128*qt + 64*(h+1))
for qt in 0..15, laid out column-sorted ascending.  This makes every
core's causal profile identical AND ideal: key-tile kt is needed exactly
by the column suffix [64*kt, 1024), so scores/den/O^T matmuls run at the
true causal width (8704 columns vs 12288 for the slot scheme) with one
uniform SPMD program.  Only the 64-wide diagonal window per key-tile
needs masking (data-driven: mask = qglob >= thr applied to exp(S)).

Each core computes the full K^T / V projections for its batch
(duplicated within the core pair - cross-core exchange was measured and
rejected: pairwise AllGather costs ~29us + 0-100us core-start stagger).

All matmul operands are bf16 (fp32 PSUM accumulation; end-to-end rel err
~3e-3 vs the 2e-2 gate).  K^T, V, Q^T, expS all live in SBUF - no DRAM
round-trips.  Everything is computed transposed so no on-chip transposes
are needed:
  stage A: K^T[e,k] = Wk-tiles.T @ Xk^T     (k processed in 2 halves)
  stage B: V[k,e]   = Xv^T-tiles.T @ Wv
  stage C: Q^T[e,q] = Wq-tiles.T @ Xq^T
  stage D per 512-col chunk: S^T[k,q-suffix] = KT-tiles.T @ Q^T,
           exp on S^T, 64-wide diagonal mask, den = ones.T @ expS
           (variable-width PSUM accumulation, widest-first), O^T[e,q] =
           V-tiles.T @ expS^T, scaled by 1/den.
Output is O^T per core in column order; the host reassembles [B,S,D].

DMA queues: bulk loads ride HWDGE (nc.sync), stores ride SWDGE
(nc.gpsimd).  Walrus accepts only ONE sync-wait per instruction, so
_split_multi_waits() splits extras onto wait-only NoOps.
"""
import sys
import types

sys.path.insert(0, "/opt/trn_rl_repo")


def _install_ntff_hook():
    try:
        import antenv
    except ImportError:
        return

    if "antenv.axon_hooks" in sys.modules:
        return
    mod = types.ModuleType("antenv.axon_hooks")
    _h = [None]
    mod.set_axon_ntff_profile_hook = lambda h: _h.__setitem__(0, h)
    mod.get_axon_ntff_profile_hook = lambda: _h[0]
    sys.modules["antenv.axon_hooks"] = mod
    antenv.axon_hooks = mod
    try:
        from trn_agent_boot.trn_boot import _ntff_profile_via_ctypes

        mod.set_axon_ntff_profile_hook(
            _ntff_profile_via_ctypes("/opt/axon/libaxon_pjrt.so"))
    except Exception:
        pass


_install_ntff_hook()


import numpy as np
import ml_dtypes
import concourse.bass as bass
import concourse.tile as tile
from concourse import mybir
from concourse.bass_utils import run_bass_kernel_spmd

P = 128
B, S, D = 4, 2048, 1024
N = 512                      # PSUM bank width / q-chunk size
NCORES = 8
SCALE = float(1.0 / np.sqrt(np.float32(2048)))

f32 = mybir.dt.float32
bf16 = mybir.dt.bfloat16
np_bf16 = ml_dtypes.bfloat16
EXP = mybir.ActivationFunctionType.Exp
MULT = mybir.AluOpType.mult
ET_GROUPS = ((0, 2), (2, 4), (4, 6), (6, 8))


def _split_multi_waits(nc):
    """Walrus allows one sync-wait per instruction; split extras onto
    wait-only NoOps inserted right before the offending instruction."""
    for f in nc.m.functions:
        for bb in f.blocks:
            insts = bb.instructions
            i = 0
            while i < len(insts):
                ins = insts[i]
                si = getattr(ins, "sync_info", None)
                if si and si.on_wait and len(si.on_wait) > 1:
                    waits = list(si.on_wait)
                    for j, w in enumerate(waits[:-1]):
                        nop = mybir.InstNoOp(
                            name=f"{ins.name}-waitsplit-{j}",
                            sync_info=mybir.SyncInfo(on_wait=[w], on_update=[]),
                            bass_nofuse=True,
                            engine=ins.engine, ins=[], outs=[])
                        insts.insert(i + j, nop)
                    i += len(waits) - 1
                    ins.sync_info = mybir.SyncInfo(
                        on_wait=[waits[-1]], on_update=list(si.on_update))
                i += 1


def build():
    nc = bass.Bass()
    # all host-side tensors are pre-arranged into SBUF layout [dp, do, cols]
    wq = nc.dram_tensor("wq", [P, 8, D], bf16, kind="ExternalInput")
    wk = nc.dram_tensor("wk", [P, 8, D], bf16, kind="ExternalInput")
    wv = nc.dram_tensor("wv", [P, 8, D], bf16, kind="ExternalInput")
    xqt = nc.dram_tensor("xqt", [P, 8, 1024], bf16, kind="ExternalInput")
    xkt = nc.dram_tensor("xkt", [P, 8, S], bf16, kind="ExternalInput")
    xvt = nc.dram_tensor("xvt", [P, 8, S], bf16, kind="ExternalInput")
    thr = nc.dram_tensor("thr", [P, 16], f32, kind="ExternalInput")
    qgl = nc.dram_tensor("qglob", [P, 1024], f32, kind="ExternalInput")
    one_in = nc.dram_tensor("ones", [P, P], bf16, kind="ExternalInput")
    out = nc.dram_tensor("out", [D, 1024], f32, kind="ExternalOutput")

    with tile.TileContext(nc) as tc:
        from contextlib import ExitStack
        with ExitStack() as ctx:
            kt_pool = ctx.enter_context(tc.tile_pool(name="ktp", bufs=1))
            v_pool = ctx.enter_context(tc.tile_pool(name="vp", bufs=1))
            qt_pool = ctx.enter_context(tc.tile_pool(name="qtp", bufs=1))
            es_pool = ctx.enter_context(tc.tile_pool(name="es", bufs=1))
            sm_pool = ctx.enter_context(tc.tile_pool(name="sm", bufs=1))
            psum = ctx.enter_context(
                tc.tile_pool(name="ps", bufs=8, space="PSUM"))

            KT = kt_pool.tile([P, 8, S], bf16)       # K^T: [e_p, e_o, k]
            V = v_pool.tile([P, 16, D], bf16)        # V:   [k_p, kt, e]
            QT = qt_pool.tile([P, 8, 1024], bf16)    # Q^T: [e_p, e_o, q_col]
            expS = es_pool.tile([P, 16, 1024], bf16)  # exp(S^T): [k_p,kt,q]

            ones = sm_pool.tile([P, P], bf16)
            nc.sync.dma_start(ones[:], one_in[:])
            qg_sb = sm_pool.tile([P, 1024], f32)
            nc.gpsimd.dma_start(qg_sb[:], qgl[:])
            thr_sb = sm_pool.tile([P, 16], f32)
            nc.gpsimd.dma_start(thr_sb[:], thr[:])

            # warm up the PE HAM clock while the first input strips stream
            # in; the operand is memset locally so warmup needs no DMA
            warm = sm_pool.tile([P, P], bf16)
            nc.vector.memset(warm[:], 1.0)
            # 80 iters bridge until the first wk/xk strips land (~13.8us)
            # with the PE fully ramped and no idle gap (idle resets p-state)
            wps = psum.tile([P, N], f32, tag="ps", name="warmps")
            for i in range(80):
                nc.tensor.matmul(wps[:, 0:P], warm[:], warm[:],
                                 start=(i == 0), stop=(i == 79))

            def copy_alt(i, dst, src):
                if i % 2 == 0:
                    nc.vector.tensor_copy(dst, src)
                else:
                    nc.scalar.copy(dst, src)

            with tc.tile_pool(name="wres", bufs=2) as w_pool, \
                    tc.tile_pool(name="xres", bufs=2) as x_pool:

                # ---- Stage A: K^T[e,k] = sum_d Wk-tiles.T @ Xk^T ----
                # wk/xk strips interleaved so the d=0 pair lands first
                wk_sb = w_pool.tile([P, 8, D], bf16, tag="w", name="wk")
                for half in range(2):
                    xk_sb = x_pool.tile([P, 8, 1024], bf16, tag="x",
                                        name=f"xk{half}")
                    for d in range(8):
                        if half == 0:
                            nc.sync.dma_start(wk_sb[:, d, :], wk[:, d, :])
                        nc.sync.dma_start(
                            xk_sb[:, d, :],
                            xkt[:, d, half * 1024:(half + 1) * 1024])
                    # kc outer: each d-strip feeds 8 back-to-back matmuls,
                    # keeping PE consumption slower than the strip DMA rate
                    for kc in range(2):
                        pss = {}
                        for et in range(8):
                            pss[et] = psum.tile(
                                [P, N], f32, tag="ps",
                                name=f"psa{half}_{kc}_{et}")
                        for d in range(8):
                            for et in range(8):
                                nc.tensor.matmul(
                                    pss[et][:],
                                    wk_sb[:, d, et * P:(et + 1) * P],
                                    xk_sb[:, d, kc * N:(kc + 1) * N],
                                    start=(d == 0), stop=(d == 7))
                        for et in range(8):
                            col = half * 1024 + kc * N
                            copy_alt(et, KT[:, et, col:col + N],
                                     pss[et][:])

                # ---- Stage B: V[k,e] = sum_d Xv^T-tiles.T @ Wv ----
                wv_sb = w_pool.tile([P, 8, D], bf16, tag="w", name="wv")
                for half in range(2):
                    xv_sb = x_pool.tile([P, 8, 1024], bf16, tag="x",
                                        name=f"xv{half}")
                    for d in range(8):
                        if half == 0:
                            nc.sync.dma_start(wv_sb[:, d, :], wv[:, d, :])
                        nc.sync.dma_start(
                            xv_sb[:, d, :],
                            xvt[:, d, half * 1024:(half + 1) * 1024])
                    for ec in range(2):
                        ps2 = {}
                        for ktl in range(8):
                            ps2[ktl] = psum.tile(
                                [P, N], f32, tag="ps",
                                name=f"psb{half}_{ec}_{ktl}")
                        for d in range(8):
                            for ktl in range(8):
                                nc.tensor.matmul(
                                    ps2[ktl][:],
                                    xv_sb[:, d, ktl * P:(ktl + 1) * P],
                                    wv_sb[:, d, ec * N:(ec + 1) * N],
                                    start=(d == 0), stop=(d == 7))
                        for ktl in range(8):
                            copy_alt(ktl,
                                     V[:, half * 8 + ktl,
                                       ec * N:(ec + 1) * N],
                                     ps2[ktl][:])

                # ---- Stage C: Q^T[e,q] = sum_d Wq-tiles.T @ Xq^T ----
                wq_sb = w_pool.tile([P, 8, D], bf16, tag="w", name="wq")
                xq_sb = x_pool.tile([P, 8, 1024], bf16, tag="x", name="xq")
                for d in range(8):
                    nc.sync.dma_start(wq_sb[:, d, :], wq[:, d, :])
                    nc.sync.dma_start(xq_sb[:, d, :], xqt[:, d, :])
                for qc in range(2):
                    psq = {}
                    for et in range(8):
                        psq[et] = psum.tile([P, N], f32, tag="ps",
                                            name=f"psq{qc}_{et}")
                    for d in range(8):
                        for et in range(8):
                            nc.tensor.matmul(
                                psq[et][:],
                                wq_sb[:, d, et * P:(et + 1) * P],
                                xq_sb[:, d, qc * N:(qc + 1) * N],
                                start=(d == 0), stop=(d == 7))
                    for et in range(8):
                        copy_alt(et, QT[:, et, qc * N:(qc + 1) * N],
                                 psq[et][:])

            # ---- Stage D: per 512-col chunk: scores, softmax, O^T ----
            # key-tile kt is needed by column suffix [64*kt, 1024)
            out_pool = ctx.enter_context(tc.tile_pool(name="op", bufs=3))
            mk_pool = ctx.enter_context(tc.tile_pool(name="mk", bufs=2))
            rd_pool = ctx.enter_context(tc.tile_pool(name="rd", bufs=2))
            for c in range(2):
                base = c * N
                kts = list(range(8 if c == 0 else 16))
                # scores^T -> exp -> diagonal mask
                for kt in kts:
                    s0 = max(0, 64 * kt - base)
                    ps = psum.tile([P, N], f32, tag="ps", name=f"pss{c}_{kt}")
                    for ec in range(8):
                        nc.tensor.matmul(
                            ps[:, s0:N], KT[:, ec, kt * P:(kt + 1) * P],
                            QT[:, ec, base + s0:base + N],
                            start=(ec == 0), stop=(ec == 7))
                    nc.scalar.activation(expS[:, kt, base + s0:base + N],
                                         ps[:, s0:N], EXP, scale=SCALE)
                    if 64 * kt // N == c:
                        w0 = 64 * kt
                        mk = mk_pool.tile([P, 64], bf16)
                        nc.vector.tensor_scalar(
                            out=mk[:], in0=qg_sb[:, w0:w0 + 64],
                            scalar1=thr_sb[:, kt:kt + 1], scalar2=None,
                            op0=mybir.AluOpType.is_ge)
                        nc.vector.tensor_tensor(
                            out=expS[:, kt, w0:w0 + 64],
                            in0=expS[:, kt, w0:w0 + 64], in1=mk[:], op=MULT)

                # denominator, replicated on all partitions
                # (variable-width accumulation: kt=0 is full width and
                # initializes the bank; later kts touch nested suffixes)
                dps = psum.tile([P, N], f32, tag="ps", name=f"dps{c}")
                for i, kt in enumerate(kts):
                    s0 = max(0, 64 * kt - base)
                    nc.tensor.matmul(dps[:, s0:N], ones[:],
                                     expS[:, kt, base + s0:base + N],
                                     start=(i == 0), stop=(i == len(kts) - 1),
                                     skip_group_check=True)
                rden = rd_pool.tile([P, N], f32)
                nc.vector.reciprocal(rden[:], dps[:])

                # O^T[e,q] from SBUF-resident V
                for et in range(8):
                    po = psum.tile([P, N], f32, tag="ps", name=f"po{c}_{et}")
                    for i, kt in enumerate(kts):
                        s0 = max(0, 64 * kt - base)
                        nc.tensor.matmul(
                            po[:, s0:N], V[:, kt, et * P:(et + 1) * P],
                            expS[:, kt, base + s0:base + N],
                            start=(i == 0), stop=(i == len(kts) - 1),
                            skip_group_check=True)
                    ot = out_pool.tile([P, N], f32)
                    nc.vector.tensor_tensor(out=ot[:], in0=po[:],
                                            in1=rden[:], op=MULT)
                    nc.gpsimd.dma_start(
                        out[et * P:(et + 1) * P, base:base + N], ot[:])

    _split_multi_waits(nc)
    return nc


_NC_CACHE = None


def _get_nc():
    global _NC_CACHE
    if _NC_CACHE is None:
        _NC_CACHE = build()
    return _NC_CACHE


def _sbufize(a):
    """[rows(1024), cols] -> [dp(128), do(8), cols] contiguous bf16."""
    r, c = a.shape
    return np.ascontiguousarray(
        a.reshape(8, P, c).transpose(1, 0, 2)).astype(np_bf16)


def _perm(h):
    """Column -> global query index for parity h (64-interleave)."""
    j = np.arange(1024)
    return 128 * (j // 64) + 64 * h + (j % 64)


def _host_prep(inputs_for_keys, inputs_for_values, inputs_for_queries,
               weight_q, weight_k, weight_v):
    f = lambda a: np.asarray(a, dtype=np.float32)
    ik, iv, iq = f(inputs_for_keys), f(inputs_for_values), f(inputs_for_queries)
    wq = _sbufize(f(weight_q))
    wk = _sbufize(f(weight_k))
    wv = _sbufize(f(weight_v))

    onesm = np.ones((P, P), np_bf16)
    p = np.arange(P, dtype=np.float32)
    thr = (128.0 * np.arange(16, dtype=np.float32))[None, :] + p[:, None]
    thr = np.ascontiguousarray(thr)          # thr[p, kt] = 128*kt + p
    in_maps = []
    for c in range(NCORES):
        b, h = c // 2, c % 2
        perm = _perm(h)
        xq = iq[b, perm]                      # [1024 cols, 1024 d]
        qglob = np.broadcast_to(perm.astype(np.float32), (P, 1024)).copy()
        in_maps.append({
            "wq": wq, "wk": wk, "wv": wv,
            "xqt": _sbufize(np.ascontiguousarray(xq.T)),
            "xkt": _sbufize(np.ascontiguousarray(ik[b].T)),
            "xvt": _sbufize(np.ascontiguousarray(iv[b].T)),
            "thr": thr, "qglob": qglob, "ones": onesm,
        })
    return in_maps


def _assemble(results):
    out = np.empty((B, S, D), np.float32)
    for c in range(NCORES):
        b, h = c // 2, c % 2
        oc = results[c]["out"].T        # [q_col, e]
        out[b, _perm(h)] = oc
    return out


def kernel(**inputs) -> np.ndarray:
    nc = _get_nc()
    in_maps = _host_prep(**inputs)
    res = run_bass_kernel_spmd(nc, in_maps, list(range(NCORES)))
    return _assemble(res.results)


def kernel_profiled(**inputs):
    """Like kernel() but also returns (output, exec_time_ns, results)."""
    nc = _get_nc()
    in_maps = _host_prep(**inputs)
    res = run_bass_kernel_spmd(nc, in_maps, list(range(NCORES)), trace=True)
    return _assemble(res.results), res.exec_time_ns, res


# revision 11
# speedup vs baseline: 1.2056x; 1.2056x over previous
"""Trainium2 Bass kernel for nn_AttentionHead (B=4, S=2048, D_IN=D_OUT=1024).

Sharding: 8 cores; core c handles batch b=c//2, parity h=c%2.  Queries are
64-interleaved: core h takes queries [128*qt + 64*h, 128*qt + 64*(h+1))
for qt in 0..15, laid out column-sorted ascending.  This makes every
core's causal profile identical AND ideal: key-tile kt is needed exactly
by the column suffix [64*kt, 1024), so scores/den/O^T matmuls run at the
true causal width (8704 columns vs 12288 for the slot scheme) with one
uniform SPMD program.  Only the 64-wide diagonal window per key-tile
needs masking (data-driven: mask = qglob >= thr applied to exp(S)).

Each core computes the full K^T / V projections for its batch
(duplicated within the core pair - cross-core exchange was measured and
rejected: pairwise AllGather costs ~29us + 0-100us core-start stagger).

All matmul operands are bf16 (fp32 PSUM accumulation; end-to-end rel err
~3e-3 vs the 2e-2 gate).  K^T, V, Q^T, expS all live in SBUF - no DRAM
round-trips.  Everything is computed transposed so no on-chip transposes
are needed:
  stage A: K^T[e,k] = Wk-tiles.T @ Xk^T     (k processed in 2 halves)
  stage B: V[k,e]   = Xv^T-tiles.T @ Wv
  stage C: Q^T[e,q] = Wq-tiles.T @ Xq^T
  stage D per 512-col chunk: S^T[k,q-suffix] = KT-tiles.T @ Q^T,
           exp on S^T, 64-wide diagonal mask, den = ones.T @ expS
           (variable-width PSUM accumulation, widest-first), O^T[e,q] =
           V-tiles.T @ expS^T, scaled by 1/den.
Output is O^T per core in column order; the host reassembles [B,S,D].

DMA queues: bulk loads ride HWDGE (nc.sync), stores ride SWDGE
(nc.gpsimd).  Walrus accepts only ONE sync-wait per instruction, so
_split_multi_waits() splits extras onto wait-only NoOps.
"""
import sys
import types

sys.path.insert(0, "/opt/trn_rl_repo")


def _install_ntff_hook():
    try:
        import antenv
    except ImportError:
        return

    if "antenv.axon_hooks" in sys.modules:
        return
    mod = types.ModuleType("antenv.axon_hooks")
    _h = [None]
    mod.set_axon_ntff_profile_hook = lambda h: _h.__setitem__(0, h)
    mod.get_axon_ntff_profile_hook = lambda: _h[0]
    sys.modules["antenv.axon_hooks"] = mod
    antenv.axon_hooks = mod
    try:
        from trn_agent_boot.trn_boot import _ntff_profile_via_ctypes

        mod.set_axon_ntff_profile_hook(
            _ntff_profile_via_ctypes("/opt/axon/libaxon_pjrt.so"))
    except Exception:
        pass


_install_ntff_hook()


import numpy as np
import ml_dtypes
import concourse.bass as bass
import concourse.tile as tile
from concourse import mybir
from concourse.bass_utils import run_bass_kernel_spmd

P = 128
B, S, D = 4, 2048, 1024
N = 512                      # PSUM bank width / q-chunk size
NCORES = 8
SCALE = float(1.0 / np.sqrt(np.float32(2048)))

f32 = mybir.dt.float32
bf16 = mybir.dt.bfloat16
np_bf16 = ml_dtypes.bfloat16
EXP = mybir.ActivationFunctionType.Exp
MULT = mybir.AluOpType.mult
ET_GROUPS = ((0, 2), (2, 4), (4, 6), (6, 8))


def _split_multi_waits(nc):
    """Walrus allows one sync-wait per instruction; split extras onto
    wait-only NoOps inserted right before the offending instruction."""
    for f in nc.m.functions:
        for bb in f.blocks:
            insts = bb.instructions
            i = 0
            while i < len(insts):
                ins = insts[i]
                si = getattr(ins, "sync_info", None)
                if si and si.on_wait and len(si.on_wait) > 1:
                    waits = list(si.on_wait)
                    for j, w in enumerate(waits[:-1]):
                        nop = mybir.InstNoOp(
                            name=f"{ins.name}-waitsplit-{j}",
                            sync_info=mybir.SyncInfo(on_wait=[w], on_update=[]),
                            bass_nofuse=True,
                            engine=ins.engine, ins=[], outs=[])
                        insts.insert(i + j, nop)
                    i += len(waits) - 1
                    ins.sync_info = mybir.SyncInfo(
                        on_wait=[waits[-1]], on_update=list(si.on_update))
                i += 1


def build():
    nc = bass.Bass()
    # all host-side tensors are pre-arranged into SBUF layout [dp, do, cols]
    wq = nc.dram_tensor("wq", [P, 8, D], bf16, kind="ExternalInput")
    wk = nc.dram_tensor("wk", [P, 8, D], bf16, kind="ExternalInput")
    wv = nc.dram_tensor("wv", [P, 8, D], bf16, kind="ExternalInput")
    xqt = nc.dram_tensor("xqt", [P, 8, 1024], bf16, kind="ExternalInput")
    xkt = nc.dram_tensor("xkt", [P, 8, S], bf16, kind="ExternalInput")
    xvt = nc.dram_tensor("xvt", [P, 8, S], bf16, kind="ExternalInput")
    thr = nc.dram_tensor("thr", [P, 16], f32, kind="ExternalInput")
    qgl = nc.dram_tensor("qglob", [P, 1024], f32, kind="ExternalInput")
    one_in = nc.dram_tensor("ones", [P, P], bf16, kind="ExternalInput")
    out = nc.dram_tensor("out", [D, 1024], f32, kind="ExternalOutput")

    with tile.TileContext(nc) as tc:
        from contextlib import ExitStack
        with ExitStack() as ctx:
            kt_pool = ctx.enter_context(tc.tile_pool(name="ktp", bufs=1))
            v_pool = ctx.enter_context(tc.tile_pool(name="vp", bufs=1))
            qt_pool = ctx.enter_context(tc.tile_pool(name="qtp", bufs=1))
            es_pool = ctx.enter_context(tc.tile_pool(name="es", bufs=1))
            sm_pool = ctx.enter_context(tc.tile_pool(name="sm", bufs=1))
            psum = ctx.enter_context(
                tc.tile_pool(name="ps", bufs=8, space="PSUM"))

            KT = kt_pool.tile([P, 8, S], bf16)       # K^T: [e_p, e_o, k]
            V = v_pool.tile([P, 16, D], bf16)        # V:   [k_p, kt, e]
            QT = qt_pool.tile([P, 8, 1024], bf16)    # Q^T: [e_p, e_o, q_col]
            expS = es_pool.tile([P, 16, 1024], bf16)  # exp(S^T): [k_p,kt,q]

            ones = sm_pool.tile([P, P], bf16)
            nc.sync.dma_start(ones[:], one_in[:])
            qg_sb = sm_pool.tile([P, 1024], f32)
            nc.gpsimd.dma_start(qg_sb[:], qgl[:])
            thr_sb = sm_pool.tile([P, 16], f32)
            nc.gpsimd.dma_start(thr_sb[:], thr[:])

            # warm up the PE clock until the first wk/xk strips land
            # (~13.8us) with no idle gap (idle resets the p-state ramp).
            # Zeros + few wide matmuls: an all-ones full-rate warmup trips
            # the power throttle and caps the whole kernel at ~2.05GHz.
            warm = sm_pool.tile([P, N], bf16)
            nc.vector.memset(warm[:], 0.0)
            wps = psum.tile([P, N], f32, tag="ps", name="warmps")
            for i in range(14):
                nc.tensor.matmul(wps[:], warm[:, 0:P], warm[:],
                                 start=(i == 0), stop=(i == 13))

            def copy_alt(i, dst, src):
                if i % 2 == 0:
                    nc.vector.tensor_copy(dst, src)
                else:
                    nc.scalar.copy(dst, src)

            with tc.tile_pool(name="wres", bufs=2) as w_pool, \
                    tc.tile_pool(name="xres", bufs=2) as x_pool:

                # ---- Stage A: K^T[e,k] = sum_d Wk-tiles.T @ Xk^T ----
                # wk/xk strips interleaved so the d=0 pair lands first
                wk_sb = w_pool.tile([P, 8, D], bf16, tag="w", name="wk")
                for half in range(2):
                    xk_sb = x_pool.tile([P, 8, 1024], bf16, tag="x",
                                        name=f"xk{half}")
                    for d in range(8):
                        if half == 0:
                            nc.sync.dma_start(wk_sb[:, d, :], wk[:, d, :])
                        nc.sync.dma_start(
                            xk_sb[:, d, :],
                            xkt[:, d, half * 1024:(half + 1) * 1024])
                    # kc outer: each d-strip feeds 8 back-to-back matmuls,
                    # keeping PE consumption slower than the strip DMA rate
                    for kc in range(2):
                        pss = {}
                        for et in range(8):
                            pss[et] = psum.tile(
                                [P, N], f32, tag="ps",
                                name=f"psa{half}_{kc}_{et}")
                        for d in range(8):
                            for et in range(8):
                                nc.tensor.matmul(
                                    pss[et][:],
                                    wk_sb[:, d, et * P:(et + 1) * P],
                                    xk_sb[:, d, kc * N:(kc + 1) * N],
                                    start=(d == 0), stop=(d == 7))
                        for et in range(8):
                            col = half * 1024 + kc * N
                            copy_alt(et, KT[:, et, col:col + N],
                                     pss[et][:])

                # ---- Stage B: V[k,e] = sum_d Xv^T-tiles.T @ Wv ----
                wv_sb = w_pool.tile([P, 8, D], bf16, tag="w", name="wv")
                for half in range(2):
                    xv_sb = x_pool.tile([P, 8, 1024], bf16, tag="x",
                                        name=f"xv{half}")
                    for d in range(8):
                        if half == 0:
                            nc.sync.dma_start(wv_sb[:, d, :], wv[:, d, :])
                        nc.sync.dma_start(
                            xv_sb[:, d, :],
                            xvt[:, d, half * 1024:(half + 1) * 1024])
                    for ec in range(2):
                        ps2 = {}
                        for ktl in range(8):
                            ps2[ktl] = psum.tile(
                                [P, N], f32, tag="ps",
                                name=f"psb{half}_{ec}_{ktl}")
                        for d in range(8):
                            for ktl in range(8):
                                nc.tensor.matmul(
                                    ps2[ktl][:],
                                    xv_sb[:, d, ktl * P:(ktl + 1) * P],
                                    wv_sb[:, d, ec * N:(ec + 1) * N],
                                    start=(d == 0), stop=(d == 7))
                        for ktl in range(8):
                            copy_alt(ktl,
                                     V[:, half * 8 + ktl,
                                       ec * N:(ec + 1) * N],
                                     ps2[ktl][:])

                # ---- Stage C: Q^T[e,q] = sum_d Wq-tiles.T @ Xq^T ----
                wq_sb = w_pool.tile([P, 8, D], bf16, tag="w", name="wq")
                xq_sb = x_pool.tile([P, 8, 1024], bf16, tag="x", name="xq")
                for d in range(8):
                    nc.sync.dma_start(wq_sb[:, d, :], wq[:, d, :])
                    nc.sync.dma_start(xq_sb[:, d, :], xqt[:, d, :])
                for qc in range(2):
                    psq = {}
                    for et in range(8):
                        psq[et] = psum.tile([P, N], f32, tag="ps",
                                            name=f"psq{qc}_{et}")
                    for d in range(8):
                        for et in range(8):
                            nc.tensor.matmul(
                                psq[et][:],
                                wq_sb[:, d, et * P:(et + 1) * P],
                                xq_sb[:, d, qc * N:(qc + 1) * N],
                                start=(d == 0), stop=(d == 7))
                    for et in range(8):
                        copy_alt(et, QT[:, et, qc * N:(qc + 1) * N],
                                 psq[et][:])

            # ---- Stage D: per 512-col chunk: scores, softmax, O^T ----
            # key-tile kt is needed by column suffix [64*kt, 1024)
            out_pool = ctx.enter_context(tc.tile_pool(name="op", bufs=3))
            mk_pool = ctx.enter_context(tc.tile_pool(name="mk", bufs=2))
            rd_pool = ctx.enter_context(tc.tile_pool(name="rd", bufs=2))
            for c in range(2):
                base = c * N
                kts = list(range(8 if c == 0 else 16))
                # scores^T -> exp -> diagonal mask
                for kt in kts:
                    s0 = max(0, 64 * kt - base)
                    ps = psum.tile([P, N], f32, tag="ps", name=f"pss{c}_{kt}")
                    for ec in range(8):
                        nc.tensor.matmul(
                            ps[:, s0:N], KT[:, ec, kt * P:(kt + 1) * P],
                            QT[:, ec, base + s0:base + N],
                            start=(ec == 0), stop=(ec == 7))
                    nc.scalar.activation(expS[:, kt, base + s0:base + N],
                                         ps[:, s0:N], EXP, scale=SCALE)
                    if 64 * kt // N == c:
                        w0 = 64 * kt
                        mk = mk_pool.tile([P, 64], bf16)
                        nc.vector.tensor_scalar(
                            out=mk[:], in0=qg_sb[:, w0:w0 + 64],
                            scalar1=thr_sb[:, kt:kt + 1], scalar2=None,
                            op0=mybir.AluOpType.is_ge)
                        nc.vector.tensor_tensor(
                            out=expS[:, kt, w0:w0 + 64],
                            in0=expS[:, kt, w0:w0 + 64], in1=mk[:], op=MULT)

                # denominator, replicated on all partitions
                # (variable-width accumulation: kt=0 is full width and
                # initializes the bank; later kts touch nested suffixes)
                dps = psum.tile([P, N], f32, tag="ps", name=f"dps{c}")
                for i, kt in enumerate(kts):
                    s0 = max(0, 64 * kt - base)
                    nc.tensor.matmul(dps[:, s0:N], ones[:],
                                     expS[:, kt, base + s0:base + N],
                                     start=(i == 0), stop=(i == len(kts) - 1),
                                     skip_group_check=True)
                rden = rd_pool.tile([P, N], f32)
                nc.vector.reciprocal(rden[:], dps[:])

                # O^T[e,q] from SBUF-resident V
                for et in range(8):
                    po = psum.tile([P, N], f32, tag="ps", name=f"po{c}_{et}")
                    for i, kt in enumerate(kts):
                        s0 = max(0, 64 * kt - base)
                        nc.tensor.matmul(
                            po[:, s0:N], V[:, kt, et * P:(et + 1) * P],
                            expS[:, kt, base + s0:base + N],
                            start=(i == 0), stop=(i == len(kts) - 1),
                            skip_group_check=True)
                    ot = out_pool.tile([P, N], f32)
                    nc.vector.tensor_tensor(out=ot[:], in0=po[:],
                                            in1=rden[:], op=MULT)
                    nc.gpsimd.dma_start(
                        out[et * P:(et + 1) * P, base:base + N], ot[:])

    _split_multi_waits(nc)
    return nc


_NC_CACHE = None


def _get_nc():
    global _NC_CACHE
    if _NC_CACHE is None:
        _NC_CACHE = build()
    return _NC_CACHE


def _sbufize(a):
    """[rows(1024), cols] -> [dp(128), do(8), cols] contiguous bf16."""
    r, c = a.shape
    return np.ascontiguousarray(
        a.reshape(8, P, c).transpose(1, 0, 2)).astype(np_bf16)


def _perm(h):
    """Column -> global query index for parity h (64-interleave)."""
    j = np.arange(1024)
    return 128 * (j // 64) + 64 * h + (j % 64)


def _host_prep(inputs_for_keys, inputs_for_values, inputs_for_queries,
               weight_q, weight_k, weight_v):
    f = lambda a: np.asarray(a, dtype=np.float32)
    ik, iv, iq = f(inputs_for_keys), f(inputs_for_values), f(inputs_for_queries)
    wq = _sbufize(f(weight_q))
    wk = _sbufize(f(weight_k))
    wv = _sbufize(f(weight_v))

    onesm = np.ones((P, P), np_bf16)
    p = np.arange(P, dtype=np.float32)
    thr = (128.0 * np.arange(16, dtype=np.float32))[None, :] + p[:, None]
    thr = np.ascontiguousarray(thr)          # thr[p, kt] = 128*kt + p
    in_maps = []
    for c in range(NCORES):
        b, h = c // 2, c % 2
        perm = _perm(h)
        xq = iq[b, perm]                      # [1024 cols, 1024 d]
        qglob = np.broadcast_to(perm.astype(np.float32), (P, 1024)).copy()
        in_maps.append({
            "wq": wq, "wk": wk, "wv": wv,
            "xqt": _sbufize(np.ascontiguousarray(xq.T)),
            "xkt": _sbufize(np.ascontiguousarray(ik[b].T)),
            "xvt": _sbufize(np.ascontiguousarray(iv[b].T)),
            "thr": thr, "qglob": qglob, "ones": onesm,
        })
    return in_maps


def _assemble(results):
    out = np.empty((B, S, D), np.float32)
    for c in range(NCORES):
        b, h = c // 2, c % 2
        oc = results[c]["out"].T        # [q_col, e]
        out[b, _perm(h)] = oc
    return out


def kernel(**inputs) -> np.ndarray:
    nc = _get_nc()
    in_maps = _host_prep(**inputs)
    res = run_bass_kernel_spmd(nc, in_maps, list(range(NCORES)))
    return _assemble(res.results)


def kernel_profiled(**inputs):
    """Like kernel() but also returns (output, exec_time_ns, results)."""
    nc = _get_nc()
    in_maps = _host_prep(**inputs)
    res = run_bass_kernel_spmd(nc, in_maps, list(range(NCORES)), trace=True)
    return _assemble(res.results), res.exec_time_ns, res


# revision 19
# speedup vs baseline: 1.4265x; 1.1832x over previous
"""Trainium2 Bass kernel for nn_AttentionHead (B=4, S=2048, D_IN=D_OUT=1024).

Sharding: 8 cores; core c handles batch b=c//2, parity h=c%2.  Queries are
64-interleaved: core h takes queries [128*qt + 64*h, 128*qt + 64*(h+1))
for qt in 0..15, laid out column-sorted ascending.  This makes every
core's causal profile identical AND ideal: key-tile kt is needed exactly
by the column suffix [64*kt, 1024), so scores/den/O^T matmuls run at the
true causal width (8704 columns vs 12288 for the slot scheme) with one
uniform SPMD program.  Only the 64-wide diagonal window per key-tile
needs masking (data-driven: mask = qglob >= thr applied to exp(S)).

Each core computes the full K^T / V projections for its batch
(duplicated within the core pair - cross-core exchange was measured and
rejected: pairwise AllGather costs ~29us + 0-100us core-start stagger).

All matmul operands are bf16 (fp32 PSUM accumulation; end-to-end rel err
~3e-3 vs the 2e-2 gate).  K^T, V, Q^T, expS all live in SBUF - no DRAM
round-trips.  Everything is computed transposed so no on-chip transposes
are needed:
  stage A: K^T[e,k] = Wk-tiles.T @ Xk^T     (k processed in 2 halves)
  stage B: V[k,e]   = Xv^T-tiles.T @ Wv
  stage C: Q^T[e,q] = Wq-tiles.T @ Xq^T
  stage D per 512-col chunk: S^T[k,q-suffix] = KT-tiles.T @ Q^T,
           exp on S^T, 64-wide diagonal mask, den = ones.T @ expS
           (variable-width PSUM accumulation, widest-first), O^T[e,q] =
           V-tiles.T @ expS^T, scaled by 1/den.
Output is O^T per core in column order; the host reassembles [B,S,D].

DMA queues: bulk loads ride HWDGE (nc.sync), stores ride SWDGE
(nc.gpsimd).  Walrus accepts only ONE sync-wait per instruction, so
_split_multi_waits() splits extras onto wait-only NoOps.
"""
import sys
import types

sys.path.insert(0, "/opt/trn_rl_repo")


def _install_ntff_hook():
    try:
        import antenv
    except ImportError:
        return

    if "antenv.axon_hooks" in sys.modules:
        return
    mod = types.ModuleType("antenv.axon_hooks")
    _h = [None]
    mod.set_axon_ntff_profile_hook = lambda h: _h.__setitem__(0, h)
    mod.get_axon_ntff_profile_hook = lambda: _h[0]
    sys.modules["antenv.axon_hooks"] = mod
    antenv.axon_hooks = mod
    try:
        from trn_agent_boot.trn_boot import _ntff_profile_via_ctypes

        mod.set_axon_ntff_profile_hook(
            _ntff_profile_via_ctypes("/opt/axon/libaxon_pjrt.so"))
    except Exception:
        pass


_install_ntff_hook()


import numpy as np
import ml_dtypes
import concourse.bass as bass
import concourse.tile as tile
from concourse import mybir
from concourse.bass_utils import run_bass_kernel_spmd

P = 128
B, S, D = 4, 2048, 1024
N = 512                      # PSUM bank width / q-chunk size
NCORES = 8
SCALE = float(1.0 / np.sqrt(np.float32(2048)))

f32 = mybir.dt.float32
bf16 = mybir.dt.bfloat16
fp8 = mybir.dt.float8e4
np_bf16 = ml_dtypes.bfloat16
np_fp8 = ml_dtypes.float8_e4m3
EXP = mybir.ActivationFunctionType.Exp
MULT = mybir.AluOpType.mult
DR = mybir.MatmulPerfMode.DoubleRow
# Wk/Wq are pre-scaled x16 on the host so their fp8 encoding avoids the
# e4m3 subnormal range; scores come out x256, absorbed into the exp scale
WSCALE = 16.0
SCALE_D = SCALE / (WSCALE * WSCALE)


def _split_multi_waits(nc):
    """Walrus allows one sync-wait per instruction; split extras onto
    wait-only NoOps inserted right before the offending instruction."""
    for f in nc.m.functions:
        for bb in f.blocks:
            insts = bb.instructions
            i = 0
            while i < len(insts):
                ins = insts[i]
                si = getattr(ins, "sync_info", None)
                if si and si.on_wait and len(si.on_wait) > 1:
                    waits = list(si.on_wait)
                    for j, w in enumerate(waits[:-1]):
                        nop = mybir.InstNoOp(
                            name=f"{ins.name}-waitsplit-{j}",
                            sync_info=mybir.SyncInfo(on_wait=[w], on_update=[]),
                            bass_nofuse=True,
                            engine=ins.engine, ins=[], outs=[])
                        insts.insert(i + j, nop)
                    i += len(waits) - 1
                    ins.sync_info = mybir.SyncInfo(
                        on_wait=[waits[-1]], on_update=list(si.on_update))
                i += 1


def build():
    nc = bass.Bass()
    # all host-side tensors are pre-arranged into SBUF layout [dp, do, cols]
    wq8 = nc.dram_tensor("wq8", [P, 4, 2, D], fp8, kind="ExternalInput")
    wk8 = nc.dram_tensor("wk8", [P, 4, 2, D], fp8, kind="ExternalInput")
    wv = nc.dram_tensor("wv", [P, 8, D], bf16, kind="ExternalInput")
    xq8 = nc.dram_tensor("xq8", [P, 4, 2, 1024], fp8, kind="ExternalInput")
    xk8 = nc.dram_tensor("xk8", [P, 4, 2, S], fp8, kind="ExternalInput")
    xvt = nc.dram_tensor("xvt", [P, 8, S], bf16, kind="ExternalInput")
    thr = nc.dram_tensor("thr", [P, 16], f32, kind="ExternalInput")
    qgl = nc.dram_tensor("qglob", [P, 1024], f32, kind="ExternalInput")
    one_in = nc.dram_tensor("ones", [P, P], bf16, kind="ExternalInput")
    out = nc.dram_tensor("out", [D, 1024], f32, kind="ExternalOutput")

    with tile.TileContext(nc) as tc:
        from contextlib import ExitStack
        with ExitStack() as ctx:
            kt_pool = ctx.enter_context(tc.tile_pool(name="ktp", bufs=1))
            v_pool = ctx.enter_context(tc.tile_pool(name="vp", bufs=1))
            qt_pool = ctx.enter_context(tc.tile_pool(name="qtp", bufs=1))
            es_pool = ctx.enter_context(tc.tile_pool(name="es", bufs=1))
            sm_pool = ctx.enter_context(tc.tile_pool(name="sm", bufs=1))
            psum = ctx.enter_context(
                tc.tile_pool(name="ps", bufs=8, space="PSUM"))

            KT = kt_pool.tile([P, 8, S], bf16)       # K^T: [e_p, e_o, k]
            V = v_pool.tile([P, 16, D], bf16)        # V:   [k_p, kt, e]
            QT = qt_pool.tile([P, 8, 1024], bf16)    # Q^T: [e_p, e_o, q_col]
            expS = es_pool.tile([P, 16, 1024], bf16)  # exp(S^T): [k_p,kt,q]

            ones = sm_pool.tile([P, P], bf16)
            nc.sync.dma_start(ones[:], one_in[:])
            qg_sb = sm_pool.tile([P, 1024], f32)
            nc.gpsimd.dma_start(qg_sb[:], qgl[:])
            thr_sb = sm_pool.tile([P, 16], f32)
            nc.gpsimd.dma_start(thr_sb[:], thr[:])

            # warm up the PE clock until the first wk/xk strips land
            # (~13.8us) with no idle gap (idle resets the p-state ramp).
            # Zeros + few wide matmuls: an all-ones full-rate warmup trips
            # the power throttle and caps the whole kernel at ~2.05GHz.
            warm = sm_pool.tile([P, N], bf16)
            nc.vector.memset(warm[:], 0.0)
            wps = psum.tile([P, N], f32, tag="ps", name="warmps")
            for i in range(14):
                nc.tensor.matmul(wps[:], warm[:, 0:P], warm[:],
                                 start=(i == 0), stop=(i == 13))

            def copy_alt(i, dst, src):
                if i % 2 == 0:
                    nc.vector.tensor_copy(dst, src)
                else:
                    nc.scalar.copy(dst, src)

            with tc.tile_pool(name="wres", bufs=2) as w_pool, \
                    tc.tile_pool(name="xres", bufs=2) as x_pool:

                # ---- Stage A: K^T[e,k] = sum_d Wk-tiles.T @ Xk^T ----
                # fp8 DoubleRow: each matmul contracts a 256-row d-pair
                # (j in 0..3), output capped at 256 cols (rhs free = 512)
                wk_sb = w_pool.tile([P, 4, 2, D], fp8, tag="w", name="wk")
                for half in range(2):
                    xk_sb = x_pool.tile([P, 4, 2, 1024], fp8, tag="x",
                                        name=f"xk{half}")
                    for j in range(4):
                        for i in range(2):
                            if half == 0:
                                nc.sync.dma_start(wk_sb[:, j, i, :],
                                                  wk8[:, j, i, :])
                            nc.sync.dma_start(
                                xk_sb[:, j, i, :],
                                xk8[:, j, i, half * 1024:(half + 1) * 1024])
                    for kc in range(2):
                        pss = {}
                        for et in range(8):
                            pss[et] = psum.tile(
                                [P, N], f32, tag="ps",
                                name=f"psa{half}_{kc}_{et}")
                        for sub in range(2):
                            c0 = kc * N + sub * 256
                            for j in range(4):
                                for et in range(8):
                                    nc.tensor.matmul(
                                        pss[et][:, sub * 256:sub * 256 + 256],
                                        wk_sb[:, j, :, et * P:(et + 1) * P],
                                        xk_sb[:, j, :, c0:c0 + 256],
                                        start=(j == 0), stop=(j == 3),
                                        perf_mode=DR)
                        for et in range(8):
                            col = half * 1024 + kc * N
                            copy_alt(et, KT[:, et, col:col + N],
                                     pss[et][:])

                # ---- Stage B: V[k,e] = sum_d Xv^T-tiles.T @ Wv ----
                wv_sb = w_pool.tile([P, 8, D], bf16, tag="w", name="wv")
                for half in range(2):
                    xv_sb = x_pool.tile([P, 8, 1024], bf16, tag="x",
                                        name=f"xv{half}")
                    for d in range(8):
                        if half == 0:
                            nc.sync.dma_start(wv_sb[:, d, :], wv[:, d, :])
                        nc.sync.dma_start(
                            xv_sb[:, d, :],
                            xvt[:, d, half * 1024:(half + 1) * 1024])
                    for ec in range(2):
                        ps2 = {}
                        for ktl in range(8):
                            ps2[ktl] = psum.tile(
                                [P, N], f32, tag="ps",
                                name=f"psb{half}_{ec}_{ktl}")
                        for d in range(8):
                            for ktl in range(8):
                                nc.tensor.matmul(
                                    ps2[ktl][:],
                                    xv_sb[:, d, ktl * P:(ktl + 1) * P],
                                    wv_sb[:, d, ec * N:(ec + 1) * N],
                                    start=(d == 0), stop=(d == 7))
                        for ktl in range(8):
                            copy_alt(ktl,
                                     V[:, half * 8 + ktl,
                                       ec * N:(ec + 1) * N],
                                     ps2[ktl][:])

                # ---- Stage C: Q^T[e,q] = sum_d Wq-tiles.T @ Xq^T ----
                wq_sb = w_pool.tile([P, 4, 2, D], fp8, tag="w", name="wq")
                xq_sb = x_pool.tile([P, 4, 2, 1024], fp8, tag="x", name="xq")
                for j in range(4):
                    for i in range(2):
                        nc.sync.dma_start(wq_sb[:, j, i, :], wq8[:, j, i, :])
                        nc.sync.dma_start(xq_sb[:, j, i, :], xq8[:, j, i, :])
                for qc in range(2):
                    psq = {}
                    for et in range(8):
                        psq[et] = psum.tile([P, N], f32, tag="ps",
                                            name=f"psq{qc}_{et}")
                    for sub in range(2):
                        c0 = qc * N + sub * 256
                        for j in range(4):
                            for et in range(8):
                                nc.tensor.matmul(
                                    psq[et][:, sub * 256:sub * 256 + 256],
                                    wq_sb[:, j, :, et * P:(et + 1) * P],
                                    xq_sb[:, j, :, c0:c0 + 256],
                                    start=(j == 0), stop=(j == 3),
                                    perf_mode=DR)
                    for et in range(8):
                        copy_alt(et, QT[:, et, qc * N:(qc + 1) * N],
                                 psq[et][:])

            # ---- Stage D: per 512-col chunk: scores, softmax, O^T ----
            # key-tile kt is needed by column suffix [64*kt, 1024)
            out_pool = ctx.enter_context(tc.tile_pool(name="op", bufs=3))
            mk_pool = ctx.enter_context(tc.tile_pool(name="mk", bufs=2))
            rd_pool = ctx.enter_context(tc.tile_pool(name="rd", bufs=2))
            for c in range(2):
                base = c * N
                kts = list(range(8 if c == 0 else 16))
                # scores^T -> exp -> diagonal mask
                for kt in kts:
                    s0 = max(0, 64 * kt - base)
                    ps = psum.tile([P, N], f32, tag="ps", name=f"pss{c}_{kt}")
                    for ec in range(8):
                        nc.tensor.matmul(
                            ps[:, s0:N], KT[:, ec, kt * P:(kt + 1) * P],
                            QT[:, ec, base + s0:base + N],
                            start=(ec == 0), stop=(ec == 7))
                    nc.scalar.activation(expS[:, kt, base + s0:base + N],
                                         ps[:, s0:N], EXP, scale=SCALE_D)
                    if 64 * kt // N == c:
                        w0 = 64 * kt
                        mk = mk_pool.tile([P, 64], bf16)
                        nc.vector.tensor_scalar(
                            out=mk[:], in0=qg_sb[:, w0:w0 + 64],
                            scalar1=thr_sb[:, kt:kt + 1], scalar2=None,
                            op0=mybir.AluOpType.is_ge)
                        nc.vector.tensor_tensor(
                            out=expS[:, kt, w0:w0 + 64],
                            in0=expS[:, kt, w0:w0 + 64], in1=mk[:], op=MULT)

                # denominator, replicated on all partitions
                # (variable-width accumulation: kt=0 is full width and
                # initializes the bank; later kts touch nested suffixes)
                dps = psum.tile([P, N], f32, tag="ps", name=f"dps{c}")
                for i, kt in enumerate(kts):
                    s0 = max(0, 64 * kt - base)
                    nc.tensor.matmul(dps[:, s0:N], ones[:],
                                     expS[:, kt, base + s0:base + N],
                                     start=(i == 0), stop=(i == len(kts) - 1),
                                     skip_group_check=True)
                rden = rd_pool.tile([P, N], f32)
                nc.vector.reciprocal(rden[:], dps[:])

                # O^T[e,q] from SBUF-resident V
                for et in range(8):
                    po = psum.tile([P, N], f32, tag="ps", name=f"po{c}_{et}")
                    for i, kt in enumerate(kts):
                        s0 = max(0, 64 * kt - base)
                        nc.tensor.matmul(
                            po[:, s0:N], V[:, kt, et * P:(et + 1) * P],
                            expS[:, kt, base + s0:base + N],
                            start=(i == 0), stop=(i == len(kts) - 1),
                            skip_group_check=True)
                    ot = out_pool.tile([P, N], f32)
                    nc.vector.tensor_tensor(out=ot[:], in0=po[:],
                                            in1=rden[:], op=MULT)
                    nc.gpsimd.dma_start(
                        out[et * P:(et + 1) * P, base:base + N], ot[:])

    _split_multi_waits(nc)
    return nc


_NC_CACHE = None


def _get_nc():
    global _NC_CACHE
    if _NC_CACHE is None:
        _NC_CACHE = build()
    return _NC_CACHE


def _sbufize(a):
    """[rows(1024), cols] -> [dp(128), do(8), cols] contiguous bf16."""
    r, c = a.shape
    return np.ascontiguousarray(
        a.reshape(8, P, c).transpose(1, 0, 2)).astype(np_bf16)


def _sbufize8(a, scale=1.0):
    """[rows(1024), cols] -> [dp(128), j(4), i(2), cols] fp8 (DoubleRow
    layout: row d = 256*j + 128*i + dp)."""
    r, c = a.shape
    return np.ascontiguousarray(
        (a * scale).reshape(4, 2, P, c).transpose(2, 0, 1, 3)).astype(np_fp8)


def _perm(h):
    """Column -> global query index for parity h (64-interleave)."""
    j = np.arange(1024)
    return 128 * (j // 64) + 64 * h + (j % 64)


def _host_prep(inputs_for_keys, inputs_for_values, inputs_for_queries,
               weight_q, weight_k, weight_v):
    f = lambda a: np.asarray(a, dtype=np.float32)
    ik, iv, iq = f(inputs_for_keys), f(inputs_for_values), f(inputs_for_queries)
    wq8 = _sbufize8(f(weight_q), WSCALE)
    wk8 = _sbufize8(f(weight_k), WSCALE)
    wv = _sbufize(f(weight_v))

    onesm = np.ones((P, P), np_bf16)
    p = np.arange(P, dtype=np.float32)
    thr = (128.0 * np.arange(16, dtype=np.float32))[None, :] + p[:, None]
    thr = np.ascontiguousarray(thr)          # thr[p, kt] = 128*kt + p
    in_maps = []
    for c in range(NCORES):
        b, h = c // 2, c % 2
        perm = _perm(h)
        xq = iq[b, perm]                      # [1024 cols, 1024 d]
        qglob = np.broadcast_to(perm.astype(np.float32), (P, 1024)).copy()
        in_maps.append({
            "wq8": wq8, "wk8": wk8, "wv": wv,
            "xq8": _sbufize8(np.ascontiguousarray(xq.T)),
            "xk8": _sbufize8(np.ascontiguousarray(ik[b].T)),
            "xvt": _sbufize(np.ascontiguousarray(iv[b].T)),
            "thr": thr, "qglob": qglob, "ones": onesm,
        })
    return in_maps


def _assemble(results):
    out = np.empty((B, S, D), np.float32)
    for c in range(NCORES):
        b, h = c // 2, c % 2
        oc = results[c]["out"].T        # [q_col, e]
        out[b, _perm(h)] = oc
    return out


def kernel(**inputs) -> np.ndarray:
    nc = _get_nc()
    in_maps = _host_prep(**inputs)
    res = run_bass_kernel_spmd(nc, in_maps, list(range(NCORES)))
    return _assemble(res.results)


def kernel_profiled(**inputs):
    """Like kernel() but also returns (output, exec_time_ns, results)."""
    nc = _get_nc()
    in_maps = _host_prep(**inputs)
    res = run_bass_kernel_spmd(nc, in_maps, list(range(NCORES)), trace=True)
    return _assemble(res.results), res.exec_time_ns, res


# revision 21
# speedup vs baseline: 1.4321x; 1.0039x over previous
"""Trainium2 Bass kernel for nn_AttentionHead (B=4, S=2048, D_IN=D_OUT=1024).

Sharding: 8 cores; core c handles batch b=c//2, parity h=c%2.  Queries are
64-interleaved: core h takes queries [128*qt + 64*h, 128*qt + 64*(h+1))
for qt in 0..15, laid out column-sorted ascending.  This makes every
core's causal profile identical AND ideal: key-tile kt is needed exactly
by the column suffix [64*kt, 1024), so scores/den/O^T matmuls run at the
true causal width (8704 columns vs 12288 for the slot scheme) with one
uniform SPMD program.  Only the 64-wide diagonal window per key-tile
needs masking (data-driven: mask = qglob >= thr applied to exp(S)).

Each core computes the full K^T / V projections for its batch
(duplicated within the core pair - cross-core exchange was measured and
rejected: pairwise AllGather costs ~29us + 0-100us core-start stagger).

All matmul operands are bf16 (fp32 PSUM accumulation; end-to-end rel err
~3e-3 vs the 2e-2 gate).  K^T, V, Q^T, expS all live in SBUF - no DRAM
round-trips.  Everything is computed transposed so no on-chip transposes
are needed:
  stage A: K^T[e,k] = Wk-tiles.T @ Xk^T     (k processed in 2 halves)
  stage B: V[k,e]   = Xv^T-tiles.T @ Wv
  stage C: Q^T[e,q] = Wq-tiles.T @ Xq^T
  stage D per 512-col chunk: S^T[k,q-suffix] = KT-tiles.T @ Q^T,
           exp on S^T, 64-wide diagonal mask, den = ones.T @ expS
           (variable-width PSUM accumulation, widest-first), O^T[e,q] =
           V-tiles.T @ expS^T, scaled by 1/den.
Output is O^T per core in column order; the host reassembles [B,S,D].

DMA queues: bulk loads ride HWDGE (nc.sync), stores ride SWDGE
(nc.gpsimd).  Walrus accepts only ONE sync-wait per instruction, so
_split_multi_waits() splits extras onto wait-only NoOps.
"""
import sys
import types

sys.path.insert(0, "/opt/trn_rl_repo")


def _install_ntff_hook():
    try:
        import antenv
    except ImportError:
        return

    if "antenv.axon_hooks" in sys.modules:
        return
    mod = types.ModuleType("antenv.axon_hooks")
    _h = [None]
    mod.set_axon_ntff_profile_hook = lambda h: _h.__setitem__(0, h)
    mod.get_axon_ntff_profile_hook = lambda: _h[0]
    sys.modules["antenv.axon_hooks"] = mod
    antenv.axon_hooks = mod
    try:
        from trn_agent_boot.trn_boot import _ntff_profile_via_ctypes

        mod.set_axon_ntff_profile_hook(
            _ntff_profile_via_ctypes("/opt/axon/libaxon_pjrt.so"))
    except Exception:
        pass


_install_ntff_hook()


import numpy as np
import ml_dtypes
import concourse.bass as bass
import concourse.tile as tile
from concourse import mybir
from concourse.bass_utils import run_bass_kernel_spmd

P = 128
B, S, D = 4, 2048, 1024
N = 512                      # PSUM bank width / q-chunk size
NCORES = 8
SCALE = float(1.0 / np.sqrt(np.float32(2048)))

f32 = mybir.dt.float32
bf16 = mybir.dt.bfloat16
fp8 = mybir.dt.float8e4
np_bf16 = ml_dtypes.bfloat16
np_fp8 = ml_dtypes.float8_e4m3
EXP = mybir.ActivationFunctionType.Exp
MULT = mybir.AluOpType.mult
DR = mybir.MatmulPerfMode.DoubleRow
# Wk/Wq are pre-scaled x16 on the host so their fp8 encoding avoids the
# e4m3 subnormal range; scores come out x256, absorbed into the exp scale
WSCALE = 16.0
SCALE_D = SCALE / (WSCALE * WSCALE)


def _split_multi_waits(nc):
    """Walrus allows one sync-wait per instruction; split extras onto
    wait-only NoOps inserted right before the offending instruction."""
    for f in nc.m.functions:
        for bb in f.blocks:
            insts = bb.instructions
            i = 0
            while i < len(insts):
                ins = insts[i]
                si = getattr(ins, "sync_info", None)
                if si and si.on_wait and len(si.on_wait) > 1:
                    waits = list(si.on_wait)
                    for j, w in enumerate(waits[:-1]):
                        nop = mybir.InstNoOp(
                            name=f"{ins.name}-waitsplit-{j}",
                            sync_info=mybir.SyncInfo(on_wait=[w], on_update=[]),
                            bass_nofuse=True,
                            engine=ins.engine, ins=[], outs=[])
                        insts.insert(i + j, nop)
                    i += len(waits) - 1
                    ins.sync_info = mybir.SyncInfo(
                        on_wait=[waits[-1]], on_update=list(si.on_update))
                i += 1


def build():
    nc = bass.Bass()
    # all host-side tensors are pre-arranged into SBUF layout [dp, do, cols]
    wq8 = nc.dram_tensor("wq8", [P, 4, 2, D], fp8, kind="ExternalInput")
    wk8 = nc.dram_tensor("wk8", [P, 4, 2, D], fp8, kind="ExternalInput")
    wv = nc.dram_tensor("wv", [P, 8, D], bf16, kind="ExternalInput")
    xq8 = nc.dram_tensor("xq8", [P, 4, 2, 1024], fp8, kind="ExternalInput")
    xk8 = nc.dram_tensor("xk8", [P, 4, 2, S], fp8, kind="ExternalInput")
    xvt = nc.dram_tensor("xvt", [P, 8, S], bf16, kind="ExternalInput")
    thr = nc.dram_tensor("thr", [P, 16], f32, kind="ExternalInput")
    qgl = nc.dram_tensor("qglob", [P, 1024], f32, kind="ExternalInput")
    one_in = nc.dram_tensor("ones", [P, P], bf16, kind="ExternalInput")
    out = nc.dram_tensor("out", [D, 1024], f32, kind="ExternalOutput")

    with tile.TileContext(nc) as tc:
        from contextlib import ExitStack
        with ExitStack() as ctx:
            kt_pool = ctx.enter_context(tc.tile_pool(name="ktp", bufs=1))
            v_pool = ctx.enter_context(tc.tile_pool(name="vp", bufs=1))
            qt_pool = ctx.enter_context(tc.tile_pool(name="qtp", bufs=1))
            es_pool = ctx.enter_context(tc.tile_pool(name="es", bufs=1))
            sm_pool = ctx.enter_context(tc.tile_pool(name="sm", bufs=1))
            psum = ctx.enter_context(
                tc.tile_pool(name="ps", bufs=8, space="PSUM"))

            KT = kt_pool.tile([P, 8, S], bf16)       # K^T: [e_p, e_o, k]
            V = v_pool.tile([P, 16, D], bf16)        # V:   [k_p, kt, e]
            QT = qt_pool.tile([P, 8, 1024], bf16)    # Q^T: [e_p, e_o, q_col]
            expS = es_pool.tile([P, 16, 1024], bf16)  # exp(S^T): [k_p,kt,q]

            ones = sm_pool.tile([P, P], bf16)
            nc.sync.dma_start(ones[:], one_in[:])
            qg_sb = sm_pool.tile([P, 1024], f32)
            nc.gpsimd.dma_start(qg_sb[:], qgl[:])
            thr_sb = sm_pool.tile([P, 16], f32)
            nc.gpsimd.dma_start(thr_sb[:], thr[:])

            # warm up the PE clock until the first wk/xk strips land
            # (~13.8us) with no idle gap (idle resets the p-state ramp).
            # Zeros + few wide matmuls: an all-ones full-rate warmup trips
            # the power throttle and caps the whole kernel at ~2.05GHz.
            warm = sm_pool.tile([P, N], bf16)
            nc.vector.memset(warm[:], 0.0)
            wps = psum.tile([P, N], f32, tag="ps", name="warmps")
            for i in range(18):
                nc.tensor.matmul(wps[:], warm[:, 0:P], warm[:],
                                 start=(i == 0), stop=(i == 17))

            def copy_alt(i, dst, src):
                if i % 2 == 0:
                    nc.vector.tensor_copy(dst, src)
                else:
                    nc.scalar.copy(dst, src)

            with tc.tile_pool(name="wres", bufs=2) as w_pool, \
                    tc.tile_pool(name="xres", bufs=2) as x_pool:

                # ---- Stage A: K^T[e,k] = sum_d Wk-tiles.T @ Xk^T ----
                # fp8 DoubleRow: each matmul contracts a 256-row d-pair
                # (j in 0..3), output capped at 256 cols (rhs free = 512)
                wk_sb = w_pool.tile([P, 4, 2, D], fp8, tag="w", name="wk")
                for half in range(2):
                    xk_sb = x_pool.tile([P, 4, 2, 1024], fp8, tag="x",
                                        name=f"xk{half}")
                    for j in range(4):
                        for i in range(2):
                            if half == 0:
                                nc.sync.dma_start(wk_sb[:, j, i, :],
                                                  wk8[:, j, i, :])
                            nc.sync.dma_start(
                                xk_sb[:, j, i, :],
                                xk8[:, j, i, half * 1024:(half + 1) * 1024])
                    for kc in range(2):
                        pss = {}
                        for et in range(8):
                            pss[et] = psum.tile(
                                [P, N], f32, tag="ps",
                                name=f"psa{half}_{kc}_{et}")
                        for sub in range(2):
                            c0 = kc * N + sub * 256
                            for j in range(4):
                                for et in range(8):
                                    nc.tensor.matmul(
                                        pss[et][:, sub * 256:sub * 256 + 256],
                                        wk_sb[:, j, :, et * P:(et + 1) * P],
                                        xk_sb[:, j, :, c0:c0 + 256],
                                        start=(j == 0), stop=(j == 3),
                                        perf_mode=DR)
                        for et in range(8):
                            col = half * 1024 + kc * N
                            copy_alt(et, KT[:, et, col:col + N],
                                     pss[et][:])

                # ---- Stage B: V[k,e] = sum_d Xv^T-tiles.T @ Wv ----
                wv_sb = w_pool.tile([P, 8, D], bf16, tag="w", name="wv")
                for half in range(2):
                    xv_sb = x_pool.tile([P, 8, 1024], bf16, tag="x",
                                        name=f"xv{half}")
                    for d in range(8):
                        if half == 0:
                            nc.sync.dma_start(wv_sb[:, d, :], wv[:, d, :])
                        nc.sync.dma_start(
                            xv_sb[:, d, :],
                            xvt[:, d, half * 1024:(half + 1) * 1024])
                    for ec in range(2):
                        ps2 = {}
                        for ktl in range(8):
                            ps2[ktl] = psum.tile(
                                [P, N], f32, tag="ps",
                                name=f"psb{half}_{ec}_{ktl}")
                        for d in range(8):
                            for ktl in range(8):
                                nc.tensor.matmul(
                                    ps2[ktl][:],
                                    xv_sb[:, d, ktl * P:(ktl + 1) * P],
                                    wv_sb[:, d, ec * N:(ec + 1) * N],
                                    start=(d == 0), stop=(d == 7))
                        for ktl in range(8):
                            copy_alt(ktl,
                                     V[:, half * 8 + ktl,
                                       ec * N:(ec + 1) * N],
                                     ps2[ktl][:])

                # ---- Stage C: Q^T[e,q] = sum_d Wq-tiles.T @ Xq^T ----
                wq_sb = w_pool.tile([P, 4, 2, D], fp8, tag="w", name="wq")
                xq_sb = x_pool.tile([P, 4, 2, 1024], fp8, tag="x", name="xq")
                for j in range(4):
                    for i in range(2):
                        nc.sync.dma_start(wq_sb[:, j, i, :], wq8[:, j, i, :])
                        nc.sync.dma_start(xq_sb[:, j, i, :], xq8[:, j, i, :])
                for qc in range(2):
                    psq = {}
                    for et in range(8):
                        psq[et] = psum.tile([P, N], f32, tag="ps",
                                            name=f"psq{qc}_{et}")
                    for sub in range(2):
                        c0 = qc * N + sub * 256
                        for j in range(4):
                            for et in range(8):
                                nc.tensor.matmul(
                                    psq[et][:, sub * 256:sub * 256 + 256],
                                    wq_sb[:, j, :, et * P:(et + 1) * P],
                                    xq_sb[:, j, :, c0:c0 + 256],
                                    start=(j == 0), stop=(j == 3),
                                    perf_mode=DR)
                    for et in range(8):
                        copy_alt(et, QT[:, et, qc * N:(qc + 1) * N],
                                 psq[et][:])

            # ---- Stage D: per 512-col chunk: scores, softmax, O^T ----
            # key-tile kt is needed by column suffix [64*kt, 1024)
            out_pool = ctx.enter_context(tc.tile_pool(name="op", bufs=3))
            mk_pool = ctx.enter_context(tc.tile_pool(name="mk", bufs=2))
            rd_pool = ctx.enter_context(tc.tile_pool(name="rd", bufs=2))
            for c in range(2):
                base = c * N
                kts = list(range(8 if c == 0 else 16))
                # scores^T -> exp -> diagonal mask
                for kt in kts:
                    s0 = max(0, 64 * kt - base)
                    ps = psum.tile([P, N], f32, tag="ps", name=f"pss{c}_{kt}")
                    for ec in range(8):
                        nc.tensor.matmul(
                            ps[:, s0:N], KT[:, ec, kt * P:(kt + 1) * P],
                            QT[:, ec, base + s0:base + N],
                            start=(ec == 0), stop=(ec == 7))
                    nc.scalar.activation(expS[:, kt, base + s0:base + N],
                                         ps[:, s0:N], EXP, scale=SCALE_D)
                    if 64 * kt // N == c:
                        w0 = 64 * kt
                        mk = mk_pool.tile([P, 64], bf16)
                        nc.vector.tensor_scalar(
                            out=mk[:], in0=qg_sb[:, w0:w0 + 64],
                            scalar1=thr_sb[:, kt:kt + 1], scalar2=None,
                            op0=mybir.AluOpType.is_ge)
                        nc.vector.tensor_tensor(
                            out=expS[:, kt, w0:w0 + 64],
                            in0=expS[:, kt, w0:w0 + 64], in1=mk[:], op=MULT)

                # denominator, replicated on all partitions
                # (variable-width accumulation: kt=0 is full width and
                # initializes the bank; later kts touch nested suffixes)
                dps = psum.tile([P, N], f32, tag="ps", name=f"dps{c}")
                for i, kt in enumerate(kts):
                    s0 = max(0, 64 * kt - base)
                    nc.tensor.matmul(dps[:, s0:N], ones[:],
                                     expS[:, kt, base + s0:base + N],
                                     start=(i == 0), stop=(i == len(kts) - 1),
                                     skip_group_check=True)
                rden = rd_pool.tile([P, N], f32)
                nc.vector.reciprocal(rden[:], dps[:])

                # O^T[e,q] from SBUF-resident V
                for et in range(8):
                    po = psum.tile([P, N], f32, tag="ps", name=f"po{c}_{et}")
                    for i, kt in enumerate(kts):
                        s0 = max(0, 64 * kt - base)
                        nc.tensor.matmul(
                            po[:, s0:N], V[:, kt, et * P:(et + 1) * P],
                            expS[:, kt, base + s0:base + N],
                            start=(i == 0), stop=(i == len(kts) - 1),
                            skip_group_check=True)
                    ot = out_pool.tile([P, N], f32)
                    nc.vector.tensor_tensor(out=ot[:], in0=po[:],
                                            in1=rden[:], op=MULT)
                    # alternate store queues: halves the final store+drain
                    eng = nc.gpsimd if et % 2 == 0 else nc.sync
                    eng.dma_start(
                        out[et * P:(et + 1) * P, base:base + N], ot[:])

    _split_multi_waits(nc)
    return nc


_NC_CACHE = None


def _get_nc():
    global _NC_CACHE
    if _NC_CACHE is None:
        _NC_CACHE = build()
    return _NC_CACHE


def _sbufize(a):
    """[rows(1024), cols] -> [dp(128), do(8), cols] contiguous bf16."""
    r, c = a.shape
    return np.ascontiguousarray(
        a.reshape(8, P, c).transpose(1, 0, 2)).astype(np_bf16)


def _sbufize8(a, scale=1.0):
    """[rows(1024), cols] -> [dp(128), j(4), i(2), cols] fp8 (DoubleRow
    layout: row d = 256*j + 128*i + dp)."""
    r, c = a.shape
    return np.ascontiguousarray(
        (a * scale).reshape(4, 2, P, c).transpose(2, 0, 1, 3)).astype(np_fp8)


def _perm(h):
    """Column -> global query index for parity h (64-interleave)."""
    j = np.arange(1024)
    return 128 * (j // 64) + 64 * h + (j % 64)


def _host_prep(inputs_for_keys, inputs_for_values, inputs_for_queries,
               weight_q, weight_k, weight_v):
    f = lambda a: np.asarray(a, dtype=np.float32)
    ik, iv, iq = f(inputs_for_keys), f(inputs_for_values), f(inputs_for_queries)
    wq8 = _sbufize8(f(weight_q), WSCALE)
    wk8 = _sbufize8(f(weight_k), WSCALE)
    wv = _sbufize(f(weight_v))

    onesm = np.ones((P, P), np_bf16)
    p = np.arange(P, dtype=np.float32)
    thr = (128.0 * np.arange(16, dtype=np.float32))[None, :] + p[:, None]
    thr = np.ascontiguousarray(thr)          # thr[p, kt] = 128*kt + p
    in_maps = []
    for c in range(NCORES):
        b, h = c // 2, c % 2
        perm = _perm(h)
        xq = iq[b, perm]                      # [1024 cols, 1024 d]
        qglob = np.broadcast_to(perm.astype(np.float32), (P, 1024)).copy()
        in_maps.append({
            "wq8": wq8, "wk8": wk8, "wv": wv,
            "xq8": _sbufize8(np.ascontiguousarray(xq.T)),
            "xk8": _sbufize8(np.ascontiguousarray(ik[b].T)),
            "xvt": _sbufize(np.ascontiguousarray(iv[b].T)),
            "thr": thr, "qglob": qglob, "ones": onesm,
        })
    return in_maps


def _assemble(results):
    out = np.empty((B, S, D), np.float32)
    for c in range(NCORES):
        b, h = c // 2, c % 2
        oc = results[c]["out"].T        # [q_col, e]
        out[b, _perm(h)] = oc
    return out


def kernel(**inputs) -> np.ndarray:
    nc = _get_nc()
    in_maps = _host_prep(**inputs)
    res = run_bass_kernel_spmd(nc, in_maps, list(range(NCORES)))
    return _assemble(res.results)


def kernel_profiled(**inputs):
    """Like kernel() but also returns (output, exec_time_ns, results)."""
    nc = _get_nc()
    in_maps = _host_prep(**inputs)
    res = run_bass_kernel_spmd(nc, in_maps, list(range(NCORES)), trace=True)
    return _assemble(res.results), res.exec_time_ns, res


# revision 23
# speedup vs baseline: 1.4457x; 1.0095x over previous
"""Trainium2 Bass kernel for nn_AttentionHead (B=4, S=2048, D_IN=D_OUT=1024).

Sharding: 8 cores; core c handles batch b=c//2, parity h=c%2.  Queries are
64-interleaved: core h takes queries [128*qt + 64*h, 128*qt + 64*(h+1))
for qt in 0..15, laid out column-sorted ascending.  This makes every
core's causal profile identical AND ideal: key-tile kt is needed exactly
by the column suffix [64*kt, 1024), so scores/den/O^T matmuls run at the
true causal width (8704 columns vs 12288 for the slot scheme) with one
uniform SPMD program.  Only the 64-wide diagonal window per key-tile
needs masking (data-driven: mask = qglob >= thr applied to exp(S)).

Each core computes the full K^T / V projections for its batch
(duplicated within the core pair - cross-core exchange was measured and
rejected: pairwise AllGather costs ~29us + 0-100us core-start stagger).

All matmul operands are bf16 (fp32 PSUM accumulation; end-to-end rel err
~3e-3 vs the 2e-2 gate).  K^T, V, Q^T, expS all live in SBUF - no DRAM
round-trips.  Everything is computed transposed so no on-chip transposes
are needed:
  stage A: K^T[e,k] = Wk-tiles.T @ Xk^T     (k processed in 2 halves)
  stage B: V[k,e]   = Xv^T-tiles.T @ Wv
  stage C: Q^T[e,q] = Wq-tiles.T @ Xq^T
  stage D per 512-col chunk: S^T[k,q-suffix] = KT-tiles.T @ Q^T,
           exp on S^T, 64-wide diagonal mask, den = ones.T @ expS
           (variable-width PSUM accumulation, widest-first), O^T[e,q] =
           V-tiles.T @ expS^T, scaled by 1/den.
Output is O^T per core in column order; the host reassembles [B,S,D].

DMA queues: bulk loads ride HWDGE (nc.sync), stores ride SWDGE
(nc.gpsimd).  Walrus accepts only ONE sync-wait per instruction, so
_split_multi_waits() splits extras onto wait-only NoOps.
"""
import sys
import types

sys.path.insert(0, "/opt/trn_rl_repo")


def _install_ntff_hook():
    try:
        import antenv
    except ImportError:
        return

    if "antenv.axon_hooks" in sys.modules:
        return
    mod = types.ModuleType("antenv.axon_hooks")
    _h = [None]
    mod.set_axon_ntff_profile_hook = lambda h: _h.__setitem__(0, h)
    mod.get_axon_ntff_profile_hook = lambda: _h[0]
    sys.modules["antenv.axon_hooks"] = mod
    antenv.axon_hooks = mod
    try:
        from trn_agent_boot.trn_boot import _ntff_profile_via_ctypes

        mod.set_axon_ntff_profile_hook(
            _ntff_profile_via_ctypes("/opt/axon/libaxon_pjrt.so"))
    except Exception:
        pass


_install_ntff_hook()


import numpy as np
import ml_dtypes
import concourse.bass as bass
import concourse.tile as tile
from concourse import mybir
from concourse.bass_utils import run_bass_kernel_spmd

P = 128
B, S, D = 4, 2048, 1024
N = 512                      # PSUM bank width / q-chunk size
NCORES = 8
SCALE = float(1.0 / np.sqrt(np.float32(2048)))

f32 = mybir.dt.float32
bf16 = mybir.dt.bfloat16
fp8 = mybir.dt.float8e4
np_bf16 = ml_dtypes.bfloat16
np_fp8 = ml_dtypes.float8_e4m3
EXP = mybir.ActivationFunctionType.Exp
MULT = mybir.AluOpType.mult
DR = mybir.MatmulPerfMode.DoubleRow
# Wk/Wq are pre-scaled x16 on the host so their fp8 encoding avoids the
# e4m3 subnormal range; scores come out x256, absorbed into the exp scale
WSCALE = 16.0
SCALE_D = SCALE / (WSCALE * WSCALE)


def _split_multi_waits(nc):
    """Walrus allows one sync-wait per instruction; split extras onto
    wait-only NoOps inserted right before the offending instruction."""
    for f in nc.m.functions:
        for bb in f.blocks:
            insts = bb.instructions
            i = 0
            while i < len(insts):
                ins = insts[i]
                si = getattr(ins, "sync_info", None)
                if si and si.on_wait and len(si.on_wait) > 1:
                    waits = list(si.on_wait)
                    for j, w in enumerate(waits[:-1]):
                        nop = mybir.InstNoOp(
                            name=f"{ins.name}-waitsplit-{j}",
                            sync_info=mybir.SyncInfo(on_wait=[w], on_update=[]),
                            bass_nofuse=True,
                            engine=ins.engine, ins=[], outs=[])
                        insts.insert(i + j, nop)
                    i += len(waits) - 1
                    ins.sync_info = mybir.SyncInfo(
                        on_wait=[waits[-1]], on_update=list(si.on_update))
                i += 1


def build():
    nc = bass.Bass()
    # all host-side tensors are pre-arranged into SBUF layout [dp, do, cols]
    wq8 = nc.dram_tensor("wq8", [P, 4, 2, D], fp8, kind="ExternalInput")
    wk8 = nc.dram_tensor("wk8", [P, 4, 2, D], fp8, kind="ExternalInput")
    wv = nc.dram_tensor("wv", [P, 8, D], bf16, kind="ExternalInput")
    xq8 = nc.dram_tensor("xq8", [P, 4, 2, 1024], fp8, kind="ExternalInput")
    xk8 = nc.dram_tensor("xk8", [P, 4, 2, S], fp8, kind="ExternalInput")
    xvt = nc.dram_tensor("xvt", [P, 8, S], bf16, kind="ExternalInput")
    thr = nc.dram_tensor("thr", [P, 16], f32, kind="ExternalInput")
    qgl = nc.dram_tensor("qglob", [P, 1024], f32, kind="ExternalInput")
    one_in = nc.dram_tensor("ones", [P, P], bf16, kind="ExternalInput")
    out = nc.dram_tensor("out", [D, 1024], f32, kind="ExternalOutput")

    with tile.TileContext(nc) as tc:
        from contextlib import ExitStack
        with ExitStack() as ctx:
            kt_pool = ctx.enter_context(tc.tile_pool(name="ktp", bufs=1))
            v_pool = ctx.enter_context(tc.tile_pool(name="vp", bufs=1))
            qt_pool = ctx.enter_context(tc.tile_pool(name="qtp", bufs=1))
            es_pool = ctx.enter_context(tc.tile_pool(name="es", bufs=1))
            sm_pool = ctx.enter_context(tc.tile_pool(name="sm", bufs=1))
            psum = ctx.enter_context(
                tc.tile_pool(name="ps", bufs=8, space="PSUM"))

            KT = kt_pool.tile([P, 8, S], bf16)       # K^T: [e_p, e_o, k]
            V = v_pool.tile([P, 16, D], bf16)        # V:   [k_p, kt, e]
            QT = qt_pool.tile([P, 8, 1024], bf16)    # Q^T: [e_p, e_o, q_col]
            expS = es_pool.tile([P, 16, 1024], bf16)  # exp(S^T): [k_p,kt,q]

            ones = sm_pool.tile([P, P], bf16)
            nc.sync.dma_start(ones[:], one_in[:])
            qg_sb = sm_pool.tile([P, 1024], f32)
            nc.gpsimd.dma_start(qg_sb[:], qgl[:])
            thr_sb = sm_pool.tile([P, 16], f32)
            nc.gpsimd.dma_start(thr_sb[:], thr[:])

            # warm up the PE clock until the first wk/xk strips land
            # (~13.8us) with no idle gap (idle resets the p-state ramp).
            # Zeros + few wide matmuls: an all-ones full-rate warmup trips
            # the power throttle and caps the whole kernel at ~2.05GHz.
            warm = sm_pool.tile([P, N], bf16)
            nc.vector.memset(warm[:], 0.0)
            # 33 iters: bridges until stage A's 2MB of fp8 strips have
            # landed (~17.7us) - early DMA only sustains ~180GB/s
            wps = psum.tile([P, N], f32, tag="ps", name="warmps")
            for i in range(33):
                nc.tensor.matmul(wps[:], warm[:, 0:P], warm[:],
                                 start=(i == 0), stop=(i == 32))

            def copy_alt(i, dst, src):
                if i % 2 == 0:
                    nc.vector.tensor_copy(dst, src)
                else:
                    nc.scalar.copy(dst, src)

            with tc.tile_pool(name="wres", bufs=2) as w_pool, \
                    tc.tile_pool(name="xres", bufs=2) as x_pool:

                # ---- Stage A: K^T[e,k] = sum_d Wk-tiles.T @ Xk^T ----
                # fp8 DoubleRow: each matmul contracts a 256-row d-pair
                # (j in 0..3), output capped at 256 cols (rhs free = 512)
                wk_sb = w_pool.tile([P, 4, 2, D], fp8, tag="w", name="wk")
                for half in range(2):
                    xk_sb = x_pool.tile([P, 4, 2, 1024], fp8, tag="x",
                                        name=f"xk{half}")
                    for j in range(4):
                        for i in range(2):
                            if half == 0:
                                nc.sync.dma_start(wk_sb[:, j, i, :],
                                                  wk8[:, j, i, :])
                            nc.sync.dma_start(
                                xk_sb[:, j, i, :],
                                xk8[:, j, i, half * 1024:(half + 1) * 1024])
                    for kc in range(2):
                        pss = {}
                        for et in range(8):
                            pss[et] = psum.tile(
                                [P, N], f32, tag="ps",
                                name=f"psa{half}_{kc}_{et}")
                        for sub in range(2):
                            c0 = kc * N + sub * 256
                            for j in range(4):
                                for et in range(8):
                                    nc.tensor.matmul(
                                        pss[et][:, sub * 256:sub * 256 + 256],
                                        wk_sb[:, j, :, et * P:(et + 1) * P],
                                        xk_sb[:, j, :, c0:c0 + 256],
                                        start=(j == 0), stop=(j == 3),
                                        perf_mode=DR)
                        for et in range(8):
                            col = half * 1024 + kc * N
                            copy_alt(et, KT[:, et, col:col + N],
                                     pss[et][:])

                # ---- Stage B: V[k,e] = sum_d Xv^T-tiles.T @ Wv ----
                wv_sb = w_pool.tile([P, 8, D], bf16, tag="w", name="wv")
                for half in range(2):
                    xv_sb = x_pool.tile([P, 8, 1024], bf16, tag="x",
                                        name=f"xv{half}")
                    for d in range(8):
                        if half == 0:
                            nc.sync.dma_start(wv_sb[:, d, :], wv[:, d, :])
                        nc.sync.dma_start(
                            xv_sb[:, d, :],
                            xvt[:, d, half * 1024:(half + 1) * 1024])
                    for ec in range(2):
                        ps2 = {}
                        for ktl in range(8):
                            ps2[ktl] = psum.tile(
                                [P, N], f32, tag="ps",
                                name=f"psb{half}_{ec}_{ktl}")
                        for d in range(8):
                            for ktl in range(8):
                                nc.tensor.matmul(
                                    ps2[ktl][:],
                                    xv_sb[:, d, ktl * P:(ktl + 1) * P],
                                    wv_sb[:, d, ec * N:(ec + 1) * N],
                                    start=(d == 0), stop=(d == 7))
                        for ktl in range(8):
                            copy_alt(ktl,
                                     V[:, half * 8 + ktl,
                                       ec * N:(ec + 1) * N],
                                     ps2[ktl][:])

                # ---- Stage C: Q^T[e,q] = sum_d Wq-tiles.T @ Xq^T ----
                wq_sb = w_pool.tile([P, 4, 2, D], fp8, tag="w", name="wq")
                xq_sb = x_pool.tile([P, 4, 2, 1024], fp8, tag="x", name="xq")
                for j in range(4):
                    for i in range(2):
                        nc.sync.dma_start(wq_sb[:, j, i, :], wq8[:, j, i, :])
                        nc.sync.dma_start(xq_sb[:, j, i, :], xq8[:, j, i, :])
                for qc in range(2):
                    psq = {}
                    for et in range(8):
                        psq[et] = psum.tile([P, N], f32, tag="ps",
                                            name=f"psq{qc}_{et}")
                    for sub in range(2):
                        c0 = qc * N + sub * 256
                        for j in range(4):
                            for et in range(8):
                                nc.tensor.matmul(
                                    psq[et][:, sub * 256:sub * 256 + 256],
                                    wq_sb[:, j, :, et * P:(et + 1) * P],
                                    xq_sb[:, j, :, c0:c0 + 256],
                                    start=(j == 0), stop=(j == 3),
                                    perf_mode=DR)
                    for et in range(8):
                        copy_alt(et, QT[:, et, qc * N:(qc + 1) * N],
                                 psq[et][:])

            # ---- Stage D: per 512-col chunk: scores, softmax, O^T ----
            # key-tile kt is needed by column suffix [64*kt, 1024)
            out_pool = ctx.enter_context(tc.tile_pool(name="op", bufs=3))
            mk_pool = ctx.enter_context(tc.tile_pool(name="mk", bufs=2))
            rd_pool = ctx.enter_context(tc.tile_pool(name="rd", bufs=2))
            for c in range(2):
                base = c * N
                kts = list(range(8 if c == 0 else 16))
                # scores^T -> exp -> diagonal mask
                for kt in kts:
                    s0 = max(0, 64 * kt - base)
                    ps = psum.tile([P, N], f32, tag="ps", name=f"pss{c}_{kt}")
                    for ec in range(8):
                        nc.tensor.matmul(
                            ps[:, s0:N], KT[:, ec, kt * P:(kt + 1) * P],
                            QT[:, ec, base + s0:base + N],
                            start=(ec == 0), stop=(ec == 7))
                    nc.scalar.activation(expS[:, kt, base + s0:base + N],
                                         ps[:, s0:N], EXP, scale=SCALE_D)
                    if 64 * kt // N == c:
                        w0 = 64 * kt
                        mk = mk_pool.tile([P, 64], bf16)
                        nc.vector.tensor_scalar(
                            out=mk[:], in0=qg_sb[:, w0:w0 + 64],
                            scalar1=thr_sb[:, kt:kt + 1], scalar2=None,
                            op0=mybir.AluOpType.is_ge)
                        nc.vector.tensor_tensor(
                            out=expS[:, kt, w0:w0 + 64],
                            in0=expS[:, kt, w0:w0 + 64], in1=mk[:], op=MULT)

                # denominator, replicated on all partitions
                # (variable-width accumulation: kt=0 is full width and
                # initializes the bank; later kts touch nested suffixes)
                dps = psum.tile([P, N], f32, tag="ps", name=f"dps{c}")
                for i, kt in enumerate(kts):
                    s0 = max(0, 64 * kt - base)
                    nc.tensor.matmul(dps[:, s0:N], ones[:],
                                     expS[:, kt, base + s0:base + N],
                                     start=(i == 0), stop=(i == len(kts) - 1),
                                     skip_group_check=True)
                rden = rd_pool.tile([P, N], f32)
                nc.vector.reciprocal(rden[:], dps[:])

                # O^T[e,q] from SBUF-resident V
                for et in range(8):
                    po = psum.tile([P, N], f32, tag="ps", name=f"po{c}_{et}")
                    for i, kt in enumerate(kts):
                        s0 = max(0, 64 * kt - base)
                        nc.tensor.matmul(
                            po[:, s0:N], V[:, kt, et * P:(et + 1) * P],
                            expS[:, kt, base + s0:base + N],
                            start=(i == 0), stop=(i == len(kts) - 1),
                            skip_group_check=True)
                    ot = out_pool.tile([P, N], f32)
                    nc.vector.tensor_tensor(out=ot[:], in0=po[:],
                                            in1=rden[:], op=MULT)
                    # stores ride sync (loads are done by stage D);
                    # keeps the gpsimd ring empty so its end-drain is short
                    nc.sync.dma_start(
                        out[et * P:(et + 1) * P, base:base + N], ot[:])

    _split_multi_waits(nc)
    return nc


_NC_CACHE = None


def _get_nc():
    global _NC_CACHE
    if _NC_CACHE is None:
        _NC_CACHE = build()
    return _NC_CACHE


def _sbufize(a):
    """[rows(1024), cols] -> [dp(128), do(8), cols] contiguous bf16."""
    r, c = a.shape
    return np.ascontiguousarray(
        a.reshape(8, P, c).transpose(1, 0, 2)).astype(np_bf16)


def _sbufize8(a, scale=1.0):
    """[rows(1024), cols] -> [dp(128), j(4), i(2), cols] fp8 (DoubleRow
    layout: row d = 256*j + 128*i + dp)."""
    r, c = a.shape
    return np.ascontiguousarray(
        (a * scale).reshape(4, 2, P, c).transpose(2, 0, 1, 3)).astype(np_fp8)


def _perm(h):
    """Column -> global query index for parity h (64-interleave)."""
    j = np.arange(1024)
    return 128 * (j // 64) + 64 * h + (j % 64)


def _host_prep(inputs_for_keys, inputs_for_values, inputs_for_queries,
               weight_q, weight_k, weight_v):
    f = lambda a: np.asarray(a, dtype=np.float32)
    ik, iv, iq = f(inputs_for_keys), f(inputs_for_values), f(inputs_for_queries)
    wq8 = _sbufize8(f(weight_q), WSCALE)
    wk8 = _sbufize8(f(weight_k), WSCALE)
    wv = _sbufize(f(weight_v))

    onesm = np.ones((P, P), np_bf16)
    p = np.arange(P, dtype=np.float32)
    thr = (128.0 * np.arange(16, dtype=np.float32))[None, :] + p[:, None]
    thr = np.ascontiguousarray(thr)          # thr[p, kt] = 128*kt + p
    in_maps = []
    for c in range(NCORES):
        b, h = c // 2, c % 2
        perm = _perm(h)
        xq = iq[b, perm]                      # [1024 cols, 1024 d]
        qglob = np.broadcast_to(perm.astype(np.float32), (P, 1024)).copy()
        in_maps.append({
            "wq8": wq8, "wk8": wk8, "wv": wv,
            "xq8": _sbufize8(np.ascontiguousarray(xq.T)),
            "xk8": _sbufize8(np.ascontiguousarray(ik[b].T)),
            "xvt": _sbufize(np.ascontiguousarray(iv[b].T)),
            "thr": thr, "qglob": qglob, "ones": onesm,
        })
    return in_maps


def _assemble(results):
    out = np.empty((B, S, D), np.float32)
    for c in range(NCORES):
        b, h = c // 2, c % 2
        oc = results[c]["out"].T        # [q_col, e]
        out[b, _perm(h)] = oc
    return out


def kernel(**inputs) -> np.ndarray:
    nc = _get_nc()
    in_maps = _host_prep(**inputs)
    res = run_bass_kernel_spmd(nc, in_maps, list(range(NCORES)))
    return _assemble(res.results)


def kernel_profiled(**inputs):
    """Like kernel() but also returns (output, exec_time_ns, results)."""
    nc = _get_nc()
    in_maps = _host_prep(**inputs)
    res = run_bass_kernel_spmd(nc, in_maps, list(range(NCORES)), trace=True)
    return _assemble(res.results), res.exec_time_ns, res


# revision 30
# speedup vs baseline: 1.4539x; 1.0057x over previous
"""Trainium2 Bass kernel for nn_AttentionHead (B=4, S=2048, D_IN=D_OUT=1024).

Sharding: 8 cores; core c handles batch b=c//2, parity h=c%2.  Queries are
64-interleaved: core h takes queries [128*qt + 64*h, 128*qt + 64*(h+1))
for qt in 0..15, laid out column-sorted ascending.  This makes every
core's causal profile identical AND ideal: key-tile kt is needed exactly
by the column suffix [64*kt, 1024), so scores/den/O^T matmuls run at the
true causal width (8704 columns vs 12288 for the slot scheme) with one
uniform SPMD program.  Only the 64-wide diagonal window per key-tile
needs masking (data-driven: mask = qglob >= thr applied to exp(S)).

Each core computes the full K^T / V projections for its batch
(duplicated within the core pair - cross-core exchange was measured and
rejected: pairwise AllGather costs ~29us + 0-100us core-start stagger).

All matmul operands are bf16 (fp32 PSUM accumulation; end-to-end rel err
~3e-3 vs the 2e-2 gate).  K^T, V, Q^T, expS all live in SBUF - no DRAM
round-trips.  Everything is computed transposed so no on-chip transposes
are needed:
  stage A: K^T[e,k] = Wk-tiles.T @ Xk^T     (k processed in 2 halves)
  stage B: V[k,e]   = Xv^T-tiles.T @ Wv
  stage C: Q^T[e,q] = Wq-tiles.T @ Xq^T
  stage D per 512-col chunk: S^T[k,q-suffix] = KT-tiles.T @ Q^T,
           exp on S^T, 64-wide diagonal mask, den = ones.T @ expS
           (variable-width PSUM accumulation, widest-first), O^T[e,q] =
           V-tiles.T @ expS^T, scaled by 1/den.
Output is O^T per core in column order; the host reassembles [B,S,D].

DMA queues: bulk loads ride HWDGE (nc.sync), stores ride SWDGE
(nc.gpsimd).  Walrus accepts only ONE sync-wait per instruction, so
_split_multi_waits() splits extras onto wait-only NoOps.
"""
import sys
import types

sys.path.insert(0, "/opt/trn_rl_repo")


def _install_ntff_hook():
    try:
        import antenv
    except ImportError:
        return

    if "antenv.axon_hooks" in sys.modules:
        return
    mod = types.ModuleType("antenv.axon_hooks")
    _h = [None]
    mod.set_axon_ntff_profile_hook = lambda h: _h.__setitem__(0, h)
    mod.get_axon_ntff_profile_hook = lambda: _h[0]
    sys.modules["antenv.axon_hooks"] = mod
    antenv.axon_hooks = mod
    try:
        from trn_agent_boot.trn_boot import _ntff_profile_via_ctypes

        mod.set_axon_ntff_profile_hook(
            _ntff_profile_via_ctypes("/opt/axon/libaxon_pjrt.so"))
    except Exception:
        pass


_install_ntff_hook()


import numpy as np
import ml_dtypes
import concourse.bass as bass
import concourse.tile as tile
from concourse import mybir
from concourse.bass_utils import run_bass_kernel_spmd

P = 128
B, S, D = 4, 2048, 1024
N = 512                      # PSUM bank width / q-chunk size
NCORES = 8
SCALE = float(1.0 / np.sqrt(np.float32(2048)))

f32 = mybir.dt.float32
bf16 = mybir.dt.bfloat16
fp8 = mybir.dt.float8e4
np_bf16 = ml_dtypes.bfloat16
np_fp8 = ml_dtypes.float8_e4m3
EXP = mybir.ActivationFunctionType.Exp
MULT = mybir.AluOpType.mult
DR = mybir.MatmulPerfMode.DoubleRow
# Wk/Wq are pre-scaled x16 on the host so their fp8 encoding avoids the
# e4m3 subnormal range; scores come out x256, absorbed into the exp scale
WSCALE = 16.0
SCALE_D = SCALE / (WSCALE * WSCALE)


def _split_multi_waits(nc):
    """Walrus allows one sync-wait per instruction; split extras onto
    wait-only NoOps inserted right before the offending instruction."""
    for f in nc.m.functions:
        for bb in f.blocks:
            insts = bb.instructions
            i = 0
            while i < len(insts):
                ins = insts[i]
                si = getattr(ins, "sync_info", None)
                if si and si.on_wait and len(si.on_wait) > 1:
                    waits = list(si.on_wait)
                    for j, w in enumerate(waits[:-1]):
                        nop = mybir.InstNoOp(
                            name=f"{ins.name}-waitsplit-{j}",
                            sync_info=mybir.SyncInfo(on_wait=[w], on_update=[]),
                            bass_nofuse=True,
                            engine=ins.engine, ins=[], outs=[])
                        insts.insert(i + j, nop)
                    i += len(waits) - 1
                    ins.sync_info = mybir.SyncInfo(
                        on_wait=[waits[-1]], on_update=list(si.on_update))
                i += 1


def build():
    nc = bass.Bass()
    # all host-side tensors are pre-arranged into SBUF layout [dp, do, cols]
    wq8 = nc.dram_tensor("wq8", [P, 4, 2, D], fp8, kind="ExternalInput")
    wk8 = nc.dram_tensor("wk8", [P, 4, 2, D], fp8, kind="ExternalInput")
    wv = nc.dram_tensor("wv", [P, 8, D], bf16, kind="ExternalInput")
    xq8 = nc.dram_tensor("xq8", [P, 4, 2, 1024], fp8, kind="ExternalInput")
    xk8 = nc.dram_tensor("xk8", [P, 4, 2, S], fp8, kind="ExternalInput")
    xvt = nc.dram_tensor("xvt", [P, 8, S], bf16, kind="ExternalInput")
    thr = nc.dram_tensor("thr", [P, 16], f32, kind="ExternalInput")
    qgl = nc.dram_tensor("qglob", [P, 1024], f32, kind="ExternalInput")
    one_in = nc.dram_tensor("ones", [P, P], bf16, kind="ExternalInput")
    out = nc.dram_tensor("out", [D, 1024], f32, kind="ExternalOutput")

    with tile.TileContext(nc) as tc:
        from contextlib import ExitStack
        with ExitStack() as ctx:
            kt_pool = ctx.enter_context(tc.tile_pool(name="ktp", bufs=1))
            v_pool = ctx.enter_context(tc.tile_pool(name="vp", bufs=1))
            qt_pool = ctx.enter_context(tc.tile_pool(name="qtp", bufs=1))
            es_pool = ctx.enter_context(tc.tile_pool(name="es", bufs=1))
            sm_pool = ctx.enter_context(tc.tile_pool(name="sm", bufs=1))
            psum = ctx.enter_context(
                tc.tile_pool(name="ps", bufs=8, space="PSUM"))

            KT = kt_pool.tile([P, 8, S], bf16)       # K^T: [e_p, e_o, k]
            V = v_pool.tile([P, 16, D], bf16)        # V:   [k_p, kt, e]
            QT = qt_pool.tile([P, 8, 1024], bf16)    # Q^T: [e_p, e_o, q_col]
            expS = es_pool.tile([P, 16, 1024], bf16)  # exp(S^T): [k_p,kt,q]

            ones = sm_pool.tile([P, P], bf16)
            nc.sync.dma_start(ones[:], one_in[:])
            qg_sb = sm_pool.tile([P, 1024], f32)
            nc.gpsimd.dma_start(qg_sb[:], qgl[:])
            thr_sb = sm_pool.tile([P, 16], f32)
            nc.gpsimd.dma_start(thr_sb[:], thr[:])

            # warm up the PE clock until the first wk/xk strips land
            # (~13.8us) with no idle gap (idle resets the p-state ramp).
            # Zeros + few wide matmuls: an all-ones full-rate warmup trips
            # the power throttle and caps the whole kernel at ~2.05GHz.
            warm = sm_pool.tile([P, N], bf16)
            nc.vector.memset(warm[:], 0.0)
            # bridges until stage A's first fp8 strips have landed
            wps = psum.tile([P, N], f32, tag="ps", name="warmps")
            for i in range(24):
                nc.tensor.matmul(wps[:], warm[:, 0:P], warm[:],
                                 start=(i == 0), stop=(i == 23))

            def copy_alt(i, dst, src):
                if i % 2 == 0:
                    nc.vector.tensor_copy(dst, src)
                else:
                    nc.scalar.copy(dst, src)

            with tc.tile_pool(name="wres", bufs=2) as w_pool, \
                    tc.tile_pool(name="xres", bufs=2) as x_pool:

                # ---- Stage A: K^T[e,k] = sum_d Wk-tiles.T @ Xk^T ----
                # fp8 DoubleRow: each matmul contracts a 256-row d-pair
                # (j in 0..3), output capped at 256 cols (rhs free = 512)
                wk_sb = w_pool.tile([P, 4, 2, D], fp8, tag="w", name="wk")
                for half in range(2):
                    xk_sb = x_pool.tile([P, 4, 2, 1024], fp8, tag="x",
                                        name=f"xk{half}")
                    for j in range(4):
                        for i in range(2):
                            if half == 0:
                                nc.sync.dma_start(wk_sb[:, j, i, :],
                                                  wk8[:, j, i, :])
                            nc.sync.dma_start(
                                xk_sb[:, j, i, :],
                                xk8[:, j, i, half * 1024:(half + 1) * 1024])
                    # j outer, subs inner: 16 matmuls per strip-set keeps
                    # PE consumption at/below the early strip arrival rate
                    for kc in range(2):
                        pss = {}
                        for et in range(8):
                            pss[et] = psum.tile(
                                [P, N], f32, tag="ps",
                                name=f"psa{half}_{kc}_{et}")
                        for j in range(4):
                            for sub in range(2):
                                c0 = kc * N + sub * 256
                                for et in range(8):
                                    nc.tensor.matmul(
                                        pss[et][:, sub * 256:sub * 256 + 256],
                                        wk_sb[:, j, :, et * P:(et + 1) * P],
                                        xk_sb[:, j, :, c0:c0 + 256],
                                        start=(j == 0 and sub == 0),
                                        stop=(j == 3 and sub == 1),
                                        perf_mode=DR)
                        for et in range(8):
                            col = half * 1024 + kc * N
                            copy_alt(et, KT[:, et, col:col + N],
                                     pss[et][:])

                # ---- Stage B: V[k,e] = sum_d Xv^T-tiles.T @ Wv ----
                wv_sb = w_pool.tile([P, 8, D], bf16, tag="w", name="wv")
                for half in range(2):
                    xv_sb = x_pool.tile([P, 8, 1024], bf16, tag="x",
                                        name=f"xv{half}")
                    for d in range(8):
                        if half == 0:
                            nc.sync.dma_start(wv_sb[:, d, :], wv[:, d, :])
                        nc.sync.dma_start(
                            xv_sb[:, d, :],
                            xvt[:, d, half * 1024:(half + 1) * 1024])
                    for ec in range(2):
                        ps2 = {}
                        for ktl in range(8):
                            ps2[ktl] = psum.tile(
                                [P, N], f32, tag="ps",
                                name=f"psb{half}_{ec}_{ktl}")
                        for d in range(8):
                            for ktl in range(8):
                                nc.tensor.matmul(
                                    ps2[ktl][:],
                                    xv_sb[:, d, ktl * P:(ktl + 1) * P],
                                    wv_sb[:, d, ec * N:(ec + 1) * N],
                                    start=(d == 0), stop=(d == 7))
                        for ktl in range(8):
                            copy_alt(ktl,
                                     V[:, half * 8 + ktl,
                                       ec * N:(ec + 1) * N],
                                     ps2[ktl][:])

                # ---- Stage C: Q^T[e,q] = sum_d Wq-tiles.T @ Xq^T ----
                wq_sb = w_pool.tile([P, 4, 2, D], fp8, tag="w", name="wq")
                xq_sb = x_pool.tile([P, 4, 2, 1024], fp8, tag="x", name="xq")
                for j in range(4):
                    for i in range(2):
                        nc.sync.dma_start(wq_sb[:, j, i, :], wq8[:, j, i, :])
                        nc.sync.dma_start(xq_sb[:, j, i, :], xq8[:, j, i, :])
                for qc in range(2):
                    psq = {}
                    for et in range(8):
                        psq[et] = psum.tile([P, N], f32, tag="ps",
                                            name=f"psq{qc}_{et}")
                    for j in range(4):
                        for sub in range(2):
                            c0 = qc * N + sub * 256
                            for et in range(8):
                                nc.tensor.matmul(
                                    psq[et][:, sub * 256:sub * 256 + 256],
                                    wq_sb[:, j, :, et * P:(et + 1) * P],
                                    xq_sb[:, j, :, c0:c0 + 256],
                                    start=(j == 0 and sub == 0),
                                    stop=(j == 3 and sub == 1),
                                    perf_mode=DR)
                    for et in range(8):
                        copy_alt(et, QT[:, et, qc * N:(qc + 1) * N],
                                 psq[et][:])

            # ---- Stage D: per 512-col chunk: scores, softmax, O^T ----
            # key-tile kt is needed by column suffix [64*kt, 1024)
            out_pool = ctx.enter_context(tc.tile_pool(name="op", bufs=3))
            mk_pool = ctx.enter_context(tc.tile_pool(name="mk", bufs=2))
            rd_pool = ctx.enter_context(tc.tile_pool(name="rd", bufs=2))
            for c in range(2):
                base = c * N
                kts = list(range(8 if c == 0 else 16))
                # scores^T -> exp -> diagonal mask
                for kt in kts:
                    s0 = max(0, 64 * kt - base)
                    ps = psum.tile([P, N], f32, tag="ps", name=f"pss{c}_{kt}")
                    for ec in range(8):
                        nc.tensor.matmul(
                            ps[:, s0:N], KT[:, ec, kt * P:(kt + 1) * P],
                            QT[:, ec, base + s0:base + N],
                            start=(ec == 0), stop=(ec == 7))
                    nc.scalar.activation(expS[:, kt, base + s0:base + N],
                                         ps[:, s0:N], EXP, scale=SCALE_D)
                    if 64 * kt // N == c:
                        w0 = 64 * kt
                        mk = mk_pool.tile([P, 64], bf16)
                        nc.vector.tensor_scalar(
                            out=mk[:], in0=qg_sb[:, w0:w0 + 64],
                            scalar1=thr_sb[:, kt:kt + 1], scalar2=None,
                            op0=mybir.AluOpType.is_ge)
                        nc.vector.tensor_tensor(
                            out=expS[:, kt, w0:w0 + 64],
                            in0=expS[:, kt, w0:w0 + 64], in1=mk[:], op=MULT)

                # denominator, replicated on all partitions
                # (variable-width accumulation: kt=0 is full width and
                # initializes the bank; later kts touch nested suffixes)
                dps = psum.tile([P, N], f32, tag="ps", name=f"dps{c}")
                for i, kt in enumerate(kts):
                    s0 = max(0, 64 * kt - base)
                    nc.tensor.matmul(dps[:, s0:N], ones[:],
                                     expS[:, kt, base + s0:base + N],
                                     start=(i == 0), stop=(i == len(kts) - 1),
                                     skip_group_check=True)
                rden = rd_pool.tile([P, N], f32)
                nc.vector.reciprocal(rden[:], dps[:])

                # O^T[e,q] from SBUF-resident V
                for et in range(8):
                    po = psum.tile([P, N], f32, tag="ps", name=f"po{c}_{et}")
                    for i, kt in enumerate(kts):
                        s0 = max(0, 64 * kt - base)
                        nc.tensor.matmul(
                            po[:, s0:N], V[:, kt, et * P:(et + 1) * P],
                            expS[:, kt, base + s0:base + N],
                            start=(i == 0), stop=(i == len(kts) - 1),
                            skip_group_check=True)
                    # mult and store split across engines/queues to halve
                    # the dependent tail after the last O^T matmul
                    ot = out_pool.tile([P, N], f32)
                    nc.vector.tensor_tensor(out=ot[:, 0:256],
                                            in0=po[:, 0:256],
                                            in1=rden[:, 0:256], op=MULT)
                    nc.vector.tensor_tensor(out=ot[:, 256:N],
                                            in0=po[:, 256:N],
                                            in1=rden[:, 256:N], op=MULT)
                    nc.sync.dma_start(
                        out[et * P:(et + 1) * P, base:base + 256],
                        ot[:, 0:256])
                    nc.gpsimd.dma_start(
                        out[et * P:(et + 1) * P, base + 256:base + N],
                        ot[:, 256:N])

    _split_multi_waits(nc)
    return nc


_NC_CACHE = None


def _get_nc():
    global _NC_CACHE
    if _NC_CACHE is None:
        _NC_CACHE = build()
    return _NC_CACHE


def _sbufize(a):
    """[rows(1024), cols] -> [dp(128), do(8), cols] contiguous bf16."""
    r, c = a.shape
    return np.ascontiguousarray(
        a.reshape(8, P, c).transpose(1, 0, 2)).astype(np_bf16)


def _sbufize8(a, scale=1.0):
    """[rows(1024), cols] -> [dp(128), j(4), i(2), cols] fp8 (DoubleRow
    layout: row d = 256*j + 128*i + dp)."""
    r, c = a.shape
    return np.ascontiguousarray(
        (a * scale).reshape(4, 2, P, c).transpose(2, 0, 1, 3)).astype(np_fp8)


def _perm(h):
    """Column -> global query index for parity h (64-interleave)."""
    j = np.arange(1024)
    return 128 * (j // 64) + 64 * h + (j % 64)


def _host_prep(inputs_for_keys, inputs_for_values, inputs_for_queries,
               weight_q, weight_k, weight_v):
    f = lambda a: np.asarray(a, dtype=np.float32)
    ik, iv, iq = f(inputs_for_keys), f(inputs_for_values), f(inputs_for_queries)
    wq8 = _sbufize8(f(weight_q), WSCALE)
    wk8 = _sbufize8(f(weight_k), WSCALE)
    wv = _sbufize(f(weight_v))

    onesm = np.ones((P, P), np_bf16)
    p = np.arange(P, dtype=np.float32)
    thr = (128.0 * np.arange(16, dtype=np.float32))[None, :] + p[:, None]
    thr = np.ascontiguousarray(thr)          # thr[p, kt] = 128*kt + p
    in_maps = []
    for c in range(NCORES):
        b, h = c // 2, c % 2
        perm = _perm(h)
        xq = iq[b, perm]                      # [1024 cols, 1024 d]
        qglob = np.broadcast_to(perm.astype(np.float32), (P, 1024)).copy()
        in_maps.append({
            "wq8": wq8, "wk8": wk8, "wv": wv,
            "xq8": _sbufize8(np.ascontiguousarray(xq.T)),
            "xk8": _sbufize8(np.ascontiguousarray(ik[b].T)),
            "xvt": _sbufize(np.ascontiguousarray(iv[b].T)),
            "thr": thr, "qglob": qglob, "ones": onesm,
        })
    return in_maps


def _assemble(results):
    out = np.empty((B, S, D), np.float32)
    for c in range(NCORES):
        b, h = c // 2, c % 2
        oc = results[c]["out"].T        # [q_col, e]
        out[b, _perm(h)] = oc
    return out


def kernel(**inputs) -> np.ndarray:
    nc = _get_nc()
    in_maps = _host_prep(**inputs)
    res = run_bass_kernel_spmd(nc, in_maps, list(range(NCORES)))
    return _assemble(res.results)


def kernel_profiled(**inputs):
    """Like kernel() but also returns (output, exec_time_ns, results)."""
    nc = _get_nc()
    in_maps = _host_prep(**inputs)
    res = run_bass_kernel_spmd(nc, in_maps, list(range(NCORES)), trace=True)
    return _assemble(res.results), res.exec_time_ns, res


# revision 31
# speedup vs baseline: 1.4646x; 1.0073x over previous
"""Trainium2 Bass kernel for nn_AttentionHead (B=4, S=2048, D_IN=D_OUT=1024).

Sharding: 8 cores; core c handles batch b=c//2, parity h=c%2.  Queries are
64-interleaved: core h takes queries [128*qt + 64*h, 128*qt + 64*(h+1))
for qt in 0..15, laid out column-sorted ascending.  This makes every
core's causal profile identical AND ideal: key-tile kt is needed exactly
by the column suffix [64*kt, 1024), so scores/den/O^T matmuls run at the
true causal width (8704 columns vs 12288 for the slot scheme) with one
uniform SPMD program.  Only the 64-wide diagonal window per key-tile
needs masking (data-driven: mask = qglob >= thr applied to exp(S)).

Each core computes the full K^T / V projections for its batch
(duplicated within the core pair - cross-core exchange was measured and
rejected: pairwise AllGather costs ~29us + 0-100us core-start stagger).

All matmul operands are bf16 (fp32 PSUM accumulation; end-to-end rel err
~3e-3 vs the 2e-2 gate).  K^T, V, Q^T, expS all live in SBUF - no DRAM
round-trips.  Everything is computed transposed so no on-chip transposes
are needed:
  stage A: K^T[e,k] = Wk-tiles.T @ Xk^T     (k processed in 2 halves)
  stage B: V[k,e]   = Xv^T-tiles.T @ Wv
  stage C: Q^T[e,q] = Wq-tiles.T @ Xq^T
  stage D per 512-col chunk: S^T[k,q-suffix] = KT-tiles.T @ Q^T,
           exp on S^T, 64-wide diagonal mask, den = ones.T @ expS
           (variable-width PSUM accumulation, widest-first), O^T[e,q] =
           V-tiles.T @ expS^T, scaled by 1/den.
Output is O^T per core in column order; the host reassembles [B,S,D].

DMA queues: bulk loads ride HWDGE (nc.sync), stores ride SWDGE
(nc.gpsimd).  Walrus accepts only ONE sync-wait per instruction, so
_split_multi_waits() splits extras onto wait-only NoOps.
"""
import sys
import types

sys.path.insert(0, "/opt/trn_rl_repo")


def _install_ntff_hook():
    try:
        import antenv
    except ImportError:
        return

    if "antenv.axon_hooks" in sys.modules:
        return
    mod = types.ModuleType("antenv.axon_hooks")
    _h = [None]
    mod.set_axon_ntff_profile_hook = lambda h: _h.__setitem__(0, h)
    mod.get_axon_ntff_profile_hook = lambda: _h[0]
    sys.modules["antenv.axon_hooks"] = mod
    antenv.axon_hooks = mod
    try:
        from trn_agent_boot.trn_boot import _ntff_profile_via_ctypes

        mod.set_axon_ntff_profile_hook(
            _ntff_profile_via_ctypes("/opt/axon/libaxon_pjrt.so"))
    except Exception:
        pass


_install_ntff_hook()


import numpy as np
import ml_dtypes
import concourse.bass as bass
import concourse.tile as tile
from concourse import mybir
from concourse.bass_utils import run_bass_kernel_spmd

P = 128
B, S, D = 4, 2048, 1024
N = 512                      # PSUM bank width / q-chunk size
NCORES = 8
SCALE = float(1.0 / np.sqrt(np.float32(2048)))

f32 = mybir.dt.float32
bf16 = mybir.dt.bfloat16
fp8 = mybir.dt.float8e4
np_bf16 = ml_dtypes.bfloat16
np_fp8 = ml_dtypes.float8_e4m3
EXP = mybir.ActivationFunctionType.Exp
MULT = mybir.AluOpType.mult
DR = mybir.MatmulPerfMode.DoubleRow
# Wk/Wq are pre-scaled x16 on the host so their fp8 encoding avoids the
# e4m3 subnormal range; scores come out x256, absorbed into the exp scale
WSCALE = 16.0
SCALE_D = SCALE / (WSCALE * WSCALE)


def _split_multi_waits(nc):
    """Walrus allows one sync-wait per instruction; split extras onto
    wait-only NoOps inserted right before the offending instruction."""
    for f in nc.m.functions:
        for bb in f.blocks:
            insts = bb.instructions
            i = 0
            while i < len(insts):
                ins = insts[i]
                si = getattr(ins, "sync_info", None)
                if si and si.on_wait and len(si.on_wait) > 1:
                    waits = list(si.on_wait)
                    for j, w in enumerate(waits[:-1]):
                        nop = mybir.InstNoOp(
                            name=f"{ins.name}-waitsplit-{j}",
                            sync_info=mybir.SyncInfo(on_wait=[w], on_update=[]),
                            bass_nofuse=True,
                            engine=ins.engine, ins=[], outs=[])
                        insts.insert(i + j, nop)
                    i += len(waits) - 1
                    ins.sync_info = mybir.SyncInfo(
                        on_wait=[waits[-1]], on_update=list(si.on_update))
                i += 1


def build():
    nc = bass.Bass()
    # all host-side tensors are pre-arranged into SBUF layout [dp, do, cols]
    wq8 = nc.dram_tensor("wq8", [P, 4, 2, D], fp8, kind="ExternalInput")
    wk8 = nc.dram_tensor("wk8", [P, 4, 2, D], fp8, kind="ExternalInput")
    wv = nc.dram_tensor("wv", [P, 8, D], bf16, kind="ExternalInput")
    xq8 = nc.dram_tensor("xq8", [P, 4, 2, 1024], fp8, kind="ExternalInput")
    xk8 = nc.dram_tensor("xk8", [P, 4, 2, S], fp8, kind="ExternalInput")
    xvt = nc.dram_tensor("xvt", [P, 8, S], bf16, kind="ExternalInput")
    thr = nc.dram_tensor("thr", [P, 16], f32, kind="ExternalInput")
    qgl = nc.dram_tensor("qglob", [P, 1024], f32, kind="ExternalInput")
    one_in = nc.dram_tensor("ones", [P, P], bf16, kind="ExternalInput")
    out = nc.dram_tensor("out", [D, 1024], f32, kind="ExternalOutput")

    with tile.TileContext(nc) as tc:
        from contextlib import ExitStack
        with ExitStack() as ctx:
            kt_pool = ctx.enter_context(tc.tile_pool(name="ktp", bufs=1))
            v_pool = ctx.enter_context(tc.tile_pool(name="vp", bufs=1))
            qt_pool = ctx.enter_context(tc.tile_pool(name="qtp", bufs=1))
            es_pool = ctx.enter_context(tc.tile_pool(name="es", bufs=1))
            sm_pool = ctx.enter_context(tc.tile_pool(name="sm", bufs=1))
            psum = ctx.enter_context(
                tc.tile_pool(name="ps", bufs=8, space="PSUM"))

            KT = kt_pool.tile([P, 8, S], bf16)       # K^T: [e_p, e_o, k]
            V = v_pool.tile([P, 16, D], bf16)        # V:   [k_p, kt, e]
            QT = qt_pool.tile([P, 8, 1024], bf16)    # Q^T: [e_p, e_o, q_col]
            expS = es_pool.tile([P, 16, 1024], bf16)  # exp(S^T): [k_p,kt,q]

            ones = sm_pool.tile([P, P], bf16)
            nc.sync.dma_start(ones[:], one_in[:])
            qg_sb = sm_pool.tile([P, 1024], f32)
            nc.gpsimd.dma_start(qg_sb[:], qgl[:])
            thr_sb = sm_pool.tile([P, 16], f32)
            nc.gpsimd.dma_start(thr_sb[:], thr[:])

            # warm up the PE clock until the first wk/xk strips land
            # (~13.8us) with no idle gap (idle resets the p-state ramp).
            # Zeros + few wide matmuls: an all-ones full-rate warmup trips
            # the power throttle and caps the whole kernel at ~2.05GHz.
            warm = sm_pool.tile([P, N], bf16)
            nc.vector.memset(warm[:], 0.0)
            # bridges until stage A's first fp8 strips have landed
            wps = psum.tile([P, N], f32, tag="ps", name="warmps")
            for i in range(21):
                nc.tensor.matmul(wps[:], warm[:, 0:P], warm[:],
                                 start=(i == 0), stop=(i == 20))

            def copy_alt(i, dst, src):
                if i % 2 == 0:
                    nc.vector.tensor_copy(dst, src)
                else:
                    nc.scalar.copy(dst, src)

            with tc.tile_pool(name="wres", bufs=2) as w_pool, \
                    tc.tile_pool(name="xres", bufs=2) as x_pool:

                # ---- Stage A: K^T[e,k] = sum_d Wk-tiles.T @ Xk^T ----
                # fp8 DoubleRow: each matmul contracts a 256-row d-pair
                # (j in 0..3), output capped at 256 cols (rhs free = 512)
                wk_sb = w_pool.tile([P, 4, 2, D], fp8, tag="w", name="wk")
                for half in range(2):
                    xk_sb = x_pool.tile([P, 4, 2, 1024], fp8, tag="x",
                                        name=f"xk{half}")
                    for j in range(4):
                        for i in range(2):
                            if half == 0:
                                nc.sync.dma_start(wk_sb[:, j, i, :],
                                                  wk8[:, j, i, :])
                            nc.sync.dma_start(
                                xk_sb[:, j, i, :],
                                xk8[:, j, i, half * 1024:(half + 1) * 1024])
                    # j outer, subs inner: 16 matmuls per strip-set keeps
                    # PE consumption at/below the early strip arrival rate
                    for kc in range(2):
                        pss = {}
                        for et in range(8):
                            pss[et] = psum.tile(
                                [P, N], f32, tag="ps",
                                name=f"psa{half}_{kc}_{et}")
                        for j in range(4):
                            for sub in range(2):
                                c0 = kc * N + sub * 256
                                for et in range(8):
                                    nc.tensor.matmul(
                                        pss[et][:, sub * 256:sub * 256 + 256],
                                        wk_sb[:, j, :, et * P:(et + 1) * P],
                                        xk_sb[:, j, :, c0:c0 + 256],
                                        start=(j == 0 and sub == 0),
                                        stop=(j == 3 and sub == 1),
                                        perf_mode=DR)
                        for et in range(8):
                            col = half * 1024 + kc * N
                            copy_alt(et, KT[:, et, col:col + N],
                                     pss[et][:])

                # ---- Stage B: V[k,e] = sum_d Xv^T-tiles.T @ Wv ----
                wv_sb = w_pool.tile([P, 8, D], bf16, tag="w", name="wv")
                for half in range(2):
                    xv_sb = x_pool.tile([P, 8, 1024], bf16, tag="x",
                                        name=f"xv{half}")
                    for d in range(8):
                        if half == 0:
                            nc.sync.dma_start(wv_sb[:, d, :], wv[:, d, :])
                        nc.sync.dma_start(
                            xv_sb[:, d, :],
                            xvt[:, d, half * 1024:(half + 1) * 1024])
                    for ec in range(2):
                        ps2 = {}
                        for ktl in range(8):
                            ps2[ktl] = psum.tile(
                                [P, N], f32, tag="ps",
                                name=f"psb{half}_{ec}_{ktl}")
                        for d in range(8):
                            for ktl in range(8):
                                nc.tensor.matmul(
                                    ps2[ktl][:],
                                    xv_sb[:, d, ktl * P:(ktl + 1) * P],
                                    wv_sb[:, d, ec * N:(ec + 1) * N],
                                    start=(d == 0), stop=(d == 7))
                        for ktl in range(8):
                            copy_alt(ktl,
                                     V[:, half * 8 + ktl,
                                       ec * N:(ec + 1) * N],
                                     ps2[ktl][:])

                # ---- Stage C: Q^T[e,q] = sum_d Wq-tiles.T @ Xq^T ----
                wq_sb = w_pool.tile([P, 4, 2, D], fp8, tag="w", name="wq")
                xq_sb = x_pool.tile([P, 4, 2, 1024], fp8, tag="x", name="xq")
                for j in range(4):
                    for i in range(2):
                        nc.sync.dma_start(wq_sb[:, j, i, :], wq8[:, j, i, :])
                        nc.sync.dma_start(xq_sb[:, j, i, :], xq8[:, j, i, :])
                for qc in range(2):
                    psq = {}
                    for et in range(8):
                        psq[et] = psum.tile([P, N], f32, tag="ps",
                                            name=f"psq{qc}_{et}")
                    for j in range(4):
                        for sub in range(2):
                            c0 = qc * N + sub * 256
                            for et in range(8):
                                nc.tensor.matmul(
                                    psq[et][:, sub * 256:sub * 256 + 256],
                                    wq_sb[:, j, :, et * P:(et + 1) * P],
                                    xq_sb[:, j, :, c0:c0 + 256],
                                    start=(j == 0 and sub == 0),
                                    stop=(j == 3 and sub == 1),
                                    perf_mode=DR)
                    for et in range(8):
                        copy_alt(et, QT[:, et, qc * N:(qc + 1) * N],
                                 psq[et][:])

            # ---- Stage D: per 512-col chunk: scores, softmax, O^T ----
            # key-tile kt is needed by column suffix [64*kt, 1024)
            out_pool = ctx.enter_context(tc.tile_pool(name="op", bufs=3))
            mk_pool = ctx.enter_context(tc.tile_pool(name="mk", bufs=2))
            rd_pool = ctx.enter_context(tc.tile_pool(name="rd", bufs=2))
            for c in range(2):
                base = c * N
                kts = list(range(8 if c == 0 else 16))
                # scores^T -> exp -> diagonal mask
                for kt in kts:
                    s0 = max(0, 64 * kt - base)
                    ps = psum.tile([P, N], f32, tag="ps", name=f"pss{c}_{kt}")
                    for ec in range(8):
                        nc.tensor.matmul(
                            ps[:, s0:N], KT[:, ec, kt * P:(kt + 1) * P],
                            QT[:, ec, base + s0:base + N],
                            start=(ec == 0), stop=(ec == 7))
                    nc.scalar.activation(expS[:, kt, base + s0:base + N],
                                         ps[:, s0:N], EXP, scale=SCALE_D)
                    if 64 * kt // N == c:
                        w0 = 64 * kt
                        mk = mk_pool.tile([P, 64], bf16)
                        nc.vector.tensor_scalar(
                            out=mk[:], in0=qg_sb[:, w0:w0 + 64],
                            scalar1=thr_sb[:, kt:kt + 1], scalar2=None,
                            op0=mybir.AluOpType.is_ge)
                        nc.vector.tensor_tensor(
                            out=expS[:, kt, w0:w0 + 64],
                            in0=expS[:, kt, w0:w0 + 64], in1=mk[:], op=MULT)

                # denominator, replicated on all partitions
                # (variable-width accumulation: kt=0 is full width and
                # initializes the bank; later kts touch nested suffixes)
                dps = psum.tile([P, N], f32, tag="ps", name=f"dps{c}")
                for i, kt in enumerate(kts):
                    s0 = max(0, 64 * kt - base)
                    nc.tensor.matmul(dps[:, s0:N], ones[:],
                                     expS[:, kt, base + s0:base + N],
                                     start=(i == 0), stop=(i == len(kts) - 1),
                                     skip_group_check=True)
                rden = rd_pool.tile([P, N], f32)
                nc.vector.reciprocal(rden[:], dps[:])

                # O^T[e,q] from SBUF-resident V
                for et in range(8):
                    po = psum.tile([P, N], f32, tag="ps", name=f"po{c}_{et}")
                    for i, kt in enumerate(kts):
                        s0 = max(0, 64 * kt - base)
                        nc.tensor.matmul(
                            po[:, s0:N], V[:, kt, et * P:(et + 1) * P],
                            expS[:, kt, base + s0:base + N],
                            start=(i == 0), stop=(i == len(kts) - 1),
                            skip_group_check=True)
                    ot = out_pool.tile([P, N], f32)
                    nc.vector.tensor_tensor(out=ot[:], in0=po[:],
                                            in1=rden[:], op=MULT)
                    # stores ride sync (loads are done by stage D); keeps
                    # the gpsimd ring idle so its end-drain overlaps compute
                    nc.sync.dma_start(
                        out[et * P:(et + 1) * P, base:base + N], ot[:])

    _split_multi_waits(nc)
    return nc


_NC_CACHE = None


def _get_nc():
    global _NC_CACHE
    if _NC_CACHE is None:
        _NC_CACHE = build()
    return _NC_CACHE


def _sbufize(a):
    """[rows(1024), cols] -> [dp(128), do(8), cols] contiguous bf16."""
    r, c = a.shape
    return np.ascontiguousarray(
        a.reshape(8, P, c).transpose(1, 0, 2)).astype(np_bf16)


def _sbufize8(a, scale=1.0):
    """[rows(1024), cols] -> [dp(128), j(4), i(2), cols] fp8 (DoubleRow
    layout: row d = 256*j + 128*i + dp)."""
    r, c = a.shape
    return np.ascontiguousarray(
        (a * scale).reshape(4, 2, P, c).transpose(2, 0, 1, 3)).astype(np_fp8)


def _perm(h):
    """Column -> global query index for parity h (64-interleave)."""
    j = np.arange(1024)
    return 128 * (j // 64) + 64 * h + (j % 64)


def _host_prep(inputs_for_keys, inputs_for_values, inputs_for_queries,
               weight_q, weight_k, weight_v):
    f = lambda a: np.asarray(a, dtype=np.float32)
    ik, iv, iq = f(inputs_for_keys), f(inputs_for_values), f(inputs_for_queries)
    wq8 = _sbufize8(f(weight_q), WSCALE)
    wk8 = _sbufize8(f(weight_k), WSCALE)
    wv = _sbufize(f(weight_v))

    onesm = np.ones((P, P), np_bf16)
    p = np.arange(P, dtype=np.float32)
    thr = (128.0 * np.arange(16, dtype=np.float32))[None, :] + p[:, None]
    thr = np.ascontiguousarray(thr)          # thr[p, kt] = 128*kt + p
    in_maps = []
    for c in range(NCORES):
        b, h = c // 2, c % 2
        perm = _perm(h)
        xq = iq[b, perm]                      # [1024 cols, 1024 d]
        qglob = np.broadcast_to(perm.astype(np.float32), (P, 1024)).copy()
        in_maps.append({
            "wq8": wq8, "wk8": wk8, "wv": wv,
            "xq8": _sbufize8(np.ascontiguousarray(xq.T)),
            "xk8": _sbufize8(np.ascontiguousarray(ik[b].T)),
            "xvt": _sbufize(np.ascontiguousarray(iv[b].T)),
            "thr": thr, "qglob": qglob, "ones": onesm,
        })
    return in_maps


def _assemble(results):
    out = np.empty((B, S, D), np.float32)
    for c in range(NCORES):
        b, h = c // 2, c % 2
        oc = results[c]["out"].T        # [q_col, e]
        out[b, _perm(h)] = oc
    return out


def kernel(**inputs) -> np.ndarray:
    nc = _get_nc()
    in_maps = _host_prep(**inputs)
    res = run_bass_kernel_spmd(nc, in_maps, list(range(NCORES)))
    return _assemble(res.results)


def kernel_profiled(**inputs):
    """Like kernel() but also returns (output, exec_time_ns, results)."""
    nc = _get_nc()
    in_maps = _host_prep(**inputs)
    res = run_bass_kernel_spmd(nc, in_maps, list(range(NCORES)), trace=True)
    return _assemble(res.results), res.exec_time_ns, res


# revision 32
# speedup vs baseline: 1.4651x; 1.0004x over previous
"""Trainium2 Bass kernel for nn_AttentionHead (B=4, S=2048, D_IN=D_OUT=1024).

Sharding: 8 cores; core c handles batch b=c//2, parity h=c%2.  Queries are
64-interleaved: core h takes queries [128*qt + 64*h, 128*qt + 64*(h+1))
for qt in 0..15, laid out column-sorted ascending.  This makes every
core's causal profile identical AND ideal: key-tile kt is needed exactly
by the column suffix [64*kt, 1024), so scores/den/O^T matmuls run at the
true causal width (8704 columns vs 12288 for the slot scheme) with one
uniform SPMD program.  Only the 64-wide diagonal window per key-tile
needs masking (data-driven: mask = qglob >= thr applied to exp(S)).

Each core computes the full K^T / V projections for its batch
(duplicated within the core pair - cross-core exchange was measured and
rejected: pairwise AllGather costs ~29us + 0-100us core-start stagger).

Dtypes: K/Q projections run in fp8-e4m3 with MatmulPerfMode.DoubleRow
(256-row contraction pairs, 2x TensorE rate); Wk/Wq are host-prescaled
x16 to dodge e4m3 subnormals, absorbed into the exp scale (/256).  The
V projection and all of stage D are bf16 (fp8 there fails the accuracy
budget).  fp32 PSUM accumulation throughout; end-to-end rel err
(max-abs/max-abs) = 9.3e-3 vs the 2e-2 gate, bit-matching the numpy
simulation of the same quantization pipeline.  K^T, V, Q^T, expS all
live in SBUF - no DRAM round-trips.  Everything is computed transposed
so no on-chip transposes are needed:
  stage A: K^T[e,k] = Wk-tiles.T @ Xk^T     (fp8 DoubleRow, k in 2 halves)
  stage B: V[k,e]   = Xv^T-tiles.T @ Wv     (bf16)
  stage C: Q^T[e,q] = Wq-tiles.T @ Xq^T     (fp8 DoubleRow)
  stage D per 512-col chunk: S^T[k,q-suffix] = KT-tiles.T @ Q^T,
           exp on S^T, 64-wide diagonal mask, den = ones.T @ expS
           (variable-width PSUM accumulation, widest-first), O^T[e,q] =
           V-tiles.T @ expS^T, scaled by 1/den.
Output is O^T per core in column order; the host reassembles [B,S,D].

Scheduling notes learned from traces:
 - PE p-state ramps over ~3us of CONTINUOUS execution and resets on any
   idle gap, so a zeros-operand warmup (memset, no DMA dependency, no
   toggle power) bridges exactly until stage A's first strips land.
   An all-ones full-rate warmup trips the power throttle and caps the
   whole kernel at ~2.05GHz (vs 2.37GHz sustained).
 - PSUM accumulation: start=True marks the whole bank pending-zero, so
   interleaved sub-region groups in one bank must carry exactly ONE
   start/stop pair (start on the first matmul, stop on the last).
 - Bulk loads ride HWDGE (nc.sync); a second DGE queue via another
   engine's dma_start corrupts data (single shared SWDGE ring).
 - Output stores ride sync too: an end-of-kernel store on gpsimd makes
   its ring drain (~3us) serialize after the last store.
Walrus accepts only ONE sync-wait per instruction, so
_split_multi_waits() splits extras onto wait-only NoOps.
"""
import sys
import types

sys.path.insert(0, "/opt/trn_rl_repo")


def _install_ntff_hook():
    try:
        import antenv
    except ImportError:
        return

    if "antenv.axon_hooks" in sys.modules:
        return
    mod = types.ModuleType("antenv.axon_hooks")
    _h = [None]
    mod.set_axon_ntff_profile_hook = lambda h: _h.__setitem__(0, h)
    mod.get_axon_ntff_profile_hook = lambda: _h[0]
    sys.modules["antenv.axon_hooks"] = mod
    antenv.axon_hooks = mod
    try:
        from trn_agent_boot.trn_boot import _ntff_profile_via_ctypes

        mod.set_axon_ntff_profile_hook(
            _ntff_profile_via_ctypes("/opt/axon/libaxon_pjrt.so"))
    except Exception:
        pass


_install_ntff_hook()


import numpy as np
import ml_dtypes
import concourse.bass as bass
import concourse.tile as tile
from concourse import mybir
from concourse.bass_utils import run_bass_kernel_spmd

P = 128
B, S, D = 4, 2048, 1024
N = 512                      # PSUM bank width / q-chunk size
NCORES = 8
SCALE = float(1.0 / np.sqrt(np.float32(2048)))

f32 = mybir.dt.float32
bf16 = mybir.dt.bfloat16
fp8 = mybir.dt.float8e4
np_bf16 = ml_dtypes.bfloat16
np_fp8 = ml_dtypes.float8_e4m3
EXP = mybir.ActivationFunctionType.Exp
MULT = mybir.AluOpType.mult
DR = mybir.MatmulPerfMode.DoubleRow
# Wk/Wq are pre-scaled x16 on the host so their fp8 encoding avoids the
# e4m3 subnormal range; scores come out x256, absorbed into the exp scale
WSCALE = 16.0
SCALE_D = SCALE / (WSCALE * WSCALE)


def _split_multi_waits(nc):
    """Walrus allows one sync-wait per instruction; split extras onto
    wait-only NoOps inserted right before the offending instruction."""
    for f in nc.m.functions:
        for bb in f.blocks:
            insts = bb.instructions
            i = 0
            while i < len(insts):
                ins = insts[i]
                si = getattr(ins, "sync_info", None)
                if si and si.on_wait and len(si.on_wait) > 1:
                    waits = list(si.on_wait)
                    for j, w in enumerate(waits[:-1]):
                        nop = mybir.InstNoOp(
                            name=f"{ins.name}-waitsplit-{j}",
                            sync_info=mybir.SyncInfo(on_wait=[w], on_update=[]),
                            bass_nofuse=True,
                            engine=ins.engine, ins=[], outs=[])
                        insts.insert(i + j, nop)
                    i += len(waits) - 1
                    ins.sync_info = mybir.SyncInfo(
                        on_wait=[waits[-1]], on_update=list(si.on_update))
                i += 1


def build():
    nc = bass.Bass()
    # all host-side tensors are pre-arranged into SBUF layout [dp, do, cols]
    wq8 = nc.dram_tensor("wq8", [P, 4, 2, D], fp8, kind="ExternalInput")
    wk8 = nc.dram_tensor("wk8", [P, 4, 2, D], fp8, kind="ExternalInput")
    wv = nc.dram_tensor("wv", [P, 8, D], bf16, kind="ExternalInput")
    xq8 = nc.dram_tensor("xq8", [P, 4, 2, 1024], fp8, kind="ExternalInput")
    xk8 = nc.dram_tensor("xk8", [P, 4, 2, S], fp8, kind="ExternalInput")
    xvt = nc.dram_tensor("xvt", [P, 8, S], bf16, kind="ExternalInput")
    thr = nc.dram_tensor("thr", [P, 16], f32, kind="ExternalInput")
    qgl = nc.dram_tensor("qglob", [P, 1024], f32, kind="ExternalInput")
    one_in = nc.dram_tensor("ones", [P, P], bf16, kind="ExternalInput")
    out = nc.dram_tensor("out", [D, 1024], f32, kind="ExternalOutput")

    with tile.TileContext(nc) as tc:
        from contextlib import ExitStack
        with ExitStack() as ctx:
            kt_pool = ctx.enter_context(tc.tile_pool(name="ktp", bufs=1))
            v_pool = ctx.enter_context(tc.tile_pool(name="vp", bufs=1))
            qt_pool = ctx.enter_context(tc.tile_pool(name="qtp", bufs=1))
            es_pool = ctx.enter_context(tc.tile_pool(name="es", bufs=1))
            sm_pool = ctx.enter_context(tc.tile_pool(name="sm", bufs=1))
            psum = ctx.enter_context(
                tc.tile_pool(name="ps", bufs=8, space="PSUM"))

            KT = kt_pool.tile([P, 8, S], bf16)       # K^T: [e_p, e_o, k]
            V = v_pool.tile([P, 16, D], bf16)        # V:   [k_p, kt, e]
            QT = qt_pool.tile([P, 8, 1024], bf16)    # Q^T: [e_p, e_o, q_col]
            expS = es_pool.tile([P, 16, 1024], bf16)  # exp(S^T): [k_p,kt,q]

            ones = sm_pool.tile([P, P], bf16)
            nc.sync.dma_start(ones[:], one_in[:])
            qg_sb = sm_pool.tile([P, 1024], f32)
            nc.gpsimd.dma_start(qg_sb[:], qgl[:])
            thr_sb = sm_pool.tile([P, 16], f32)
            nc.gpsimd.dma_start(thr_sb[:], thr[:])

            # warm up the PE clock until the first wk/xk strips land
            # (~13.8us) with no idle gap (idle resets the p-state ramp).
            # Zeros + few wide matmuls: an all-ones full-rate warmup trips
            # the power throttle and caps the whole kernel at ~2.05GHz.
            warm = sm_pool.tile([P, N], bf16)
            nc.vector.memset(warm[:], 0.0)
            # bridges until stage A's first fp8 strips have landed
            wps = psum.tile([P, N], f32, tag="ps", name="warmps")
            for i in range(21):
                nc.tensor.matmul(wps[:], warm[:, 0:P], warm[:],
                                 start=(i == 0), stop=(i == 20))

            def copy_alt(i, dst, src):
                if i % 2 == 0:
                    nc.vector.tensor_copy(dst, src)
                else:
                    nc.scalar.copy(dst, src)

            with tc.tile_pool(name="wres", bufs=2) as w_pool, \
                    tc.tile_pool(name="xres", bufs=2) as x_pool:

                # ---- Stage A: K^T[e,k] = sum_d Wk-tiles.T @ Xk^T ----
                # fp8 DoubleRow: each matmul contracts a 256-row d-pair
                # (j in 0..3), output capped at 256 cols (rhs free = 512)
                wk_sb = w_pool.tile([P, 4, 2, D], fp8, tag="w", name="wk")
                for half in range(2):
                    xk_sb = x_pool.tile([P, 4, 2, 1024], fp8, tag="x",
                                        name=f"xk{half}")
                    for j in range(4):
                        for i in range(2):
                            if half == 0:
                                nc.sync.dma_start(wk_sb[:, j, i, :],
                                                  wk8[:, j, i, :])
                            nc.sync.dma_start(
                                xk_sb[:, j, i, :],
                                xk8[:, j, i, half * 1024:(half + 1) * 1024])
                    # j outer, subs inner: 16 matmuls per strip-set keeps
                    # PE consumption at/below the early strip arrival rate
                    for kc in range(2):
                        pss = {}
                        for et in range(8):
                            pss[et] = psum.tile(
                                [P, N], f32, tag="ps",
                                name=f"psa{half}_{kc}_{et}")
                        for j in range(4):
                            for sub in range(2):
                                c0 = kc * N + sub * 256
                                for et in range(8):
                                    nc.tensor.matmul(
                                        pss[et][:, sub * 256:sub * 256 + 256],
                                        wk_sb[:, j, :, et * P:(et + 1) * P],
                                        xk_sb[:, j, :, c0:c0 + 256],
                                        start=(j == 0 and sub == 0),
                                        stop=(j == 3 and sub == 1),
                                        perf_mode=DR)
                        for et in range(8):
                            col = half * 1024 + kc * N
                            copy_alt(et, KT[:, et, col:col + N],
                                     pss[et][:])

                # ---- Stage B: V[k,e] = sum_d Xv^T-tiles.T @ Wv ----
                wv_sb = w_pool.tile([P, 8, D], bf16, tag="w", name="wv")
                for half in range(2):
                    xv_sb = x_pool.tile([P, 8, 1024], bf16, tag="x",
                                        name=f"xv{half}")
                    for d in range(8):
                        if half == 0:
                            nc.sync.dma_start(wv_sb[:, d, :], wv[:, d, :])
                        nc.sync.dma_start(
                            xv_sb[:, d, :],
                            xvt[:, d, half * 1024:(half + 1) * 1024])
                    for ec in range(2):
                        ps2 = {}
                        for ktl in range(8):
                            ps2[ktl] = psum.tile(
                                [P, N], f32, tag="ps",
                                name=f"psb{half}_{ec}_{ktl}")
                        for d in range(8):
                            for ktl in range(8):
                                nc.tensor.matmul(
                                    ps2[ktl][:],
                                    xv_sb[:, d, ktl * P:(ktl + 1) * P],
                                    wv_sb[:, d, ec * N:(ec + 1) * N],
                                    start=(d == 0), stop=(d == 7))
                        for ktl in range(8):
                            copy_alt(ktl,
                                     V[:, half * 8 + ktl,
                                       ec * N:(ec + 1) * N],
                                     ps2[ktl][:])

                # ---- Stage C: Q^T[e,q] = sum_d Wq-tiles.T @ Xq^T ----
                wq_sb = w_pool.tile([P, 4, 2, D], fp8, tag="w", name="wq")
                xq_sb = x_pool.tile([P, 4, 2, 1024], fp8, tag="x", name="xq")
                for j in range(4):
                    for i in range(2):
                        nc.sync.dma_start(wq_sb[:, j, i, :], wq8[:, j, i, :])
                        nc.sync.dma_start(xq_sb[:, j, i, :], xq8[:, j, i, :])
                for qc in range(2):
                    psq = {}
                    for et in range(8):
                        psq[et] = psum.tile([P, N], f32, tag="ps",
                                            name=f"psq{qc}_{et}")
                    for j in range(4):
                        for sub in range(2):
                            c0 = qc * N + sub * 256
                            for et in range(8):
                                nc.tensor.matmul(
                                    psq[et][:, sub * 256:sub * 256 + 256],
                                    wq_sb[:, j, :, et * P:(et + 1) * P],
                                    xq_sb[:, j, :, c0:c0 + 256],
                                    start=(j == 0 and sub == 0),
                                    stop=(j == 3 and sub == 1),
                                    perf_mode=DR)
                    for et in range(8):
                        copy_alt(et, QT[:, et, qc * N:(qc + 1) * N],
                                 psq[et][:])

            # ---- Stage D: per 512-col chunk: scores, softmax, O^T ----
            # key-tile kt is needed by column suffix [64*kt, 1024)
            out_pool = ctx.enter_context(tc.tile_pool(name="op", bufs=3))
            mk_pool = ctx.enter_context(tc.tile_pool(name="mk", bufs=2))
            rd_pool = ctx.enter_context(tc.tile_pool(name="rd", bufs=2))
            for c in range(2):
                base = c * N
                kts = list(range(8 if c == 0 else 16))
                # scores^T -> exp -> diagonal mask
                for kt in kts:
                    s0 = max(0, 64 * kt - base)
                    ps = psum.tile([P, N], f32, tag="ps", name=f"pss{c}_{kt}")
                    for ec in range(8):
                        nc.tensor.matmul(
                            ps[:, s0:N], KT[:, ec, kt * P:(kt + 1) * P],
                            QT[:, ec, base + s0:base + N],
                            start=(ec == 0), stop=(ec == 7))
                    nc.scalar.activation(expS[:, kt, base + s0:base + N],
                                         ps[:, s0:N], EXP, scale=SCALE_D)
                    if 64 * kt // N == c:
                        w0 = 64 * kt
                        mk = mk_pool.tile([P, 64], bf16)
                        nc.vector.tensor_scalar(
                            out=mk[:], in0=qg_sb[:, w0:w0 + 64],
                            scalar1=thr_sb[:, kt:kt + 1], scalar2=None,
                            op0=mybir.AluOpType.is_ge)
                        nc.vector.tensor_tensor(
                            out=expS[:, kt, w0:w0 + 64],
                            in0=expS[:, kt, w0:w0 + 64], in1=mk[:], op=MULT)

                # denominator, replicated on all partitions
                # (variable-width accumulation: kt=0 is full width and
                # initializes the bank; later kts touch nested suffixes)
                dps = psum.tile([P, N], f32, tag="ps", name=f"dps{c}")
                for i, kt in enumerate(kts):
                    s0 = max(0, 64 * kt - base)
                    nc.tensor.matmul(dps[:, s0:N], ones[:],
                                     expS[:, kt, base + s0:base + N],
                                     start=(i == 0), stop=(i == len(kts) - 1),
                                     skip_group_check=True)
                rden = rd_pool.tile([P, N], f32)
                nc.vector.reciprocal(rden[:], dps[:])

                # O^T[e,q] from SBUF-resident V
                for et in range(8):
                    po = psum.tile([P, N], f32, tag="ps", name=f"po{c}_{et}")
                    for i, kt in enumerate(kts):
                        s0 = max(0, 64 * kt - base)
                        nc.tensor.matmul(
                            po[:, s0:N], V[:, kt, et * P:(et + 1) * P],
                            expS[:, kt, base + s0:base + N],
                            start=(i == 0), stop=(i == len(kts) - 1),
                            skip_group_check=True)
                    ot = out_pool.tile([P, N], f32)
                    nc.vector.tensor_tensor(out=ot[:], in0=po[:],
                                            in1=rden[:], op=MULT)
                    # stores ride sync (loads are done by stage D); keeps
                    # the gpsimd ring idle so its end-drain overlaps compute
                    nc.sync.dma_start(
                        out[et * P:(et + 1) * P, base:base + N], ot[:])

    _split_multi_waits(nc)
    return nc


_NC_CACHE = None


def _get_nc():
    global _NC_CACHE
    if _NC_CACHE is None:
        _NC_CACHE = build()
    return _NC_CACHE


def _sbufize(a):
    """[rows(1024), cols] -> [dp(128), do(8), cols] contiguous bf16."""
    r, c = a.shape
    return np.ascontiguousarray(
        a.reshape(8, P, c).transpose(1, 0, 2)).astype(np_bf16)


def _sbufize8(a, scale=1.0):
    """[rows(1024), cols] -> [dp(128), j(4), i(2), cols] fp8 (DoubleRow
    layout: row d = 256*j + 128*i + dp)."""
    r, c = a.shape
    return np.ascontiguousarray(
        (a * scale).reshape(4, 2, P, c).transpose(2, 0, 1, 3)).astype(np_fp8)


def _perm(h):
    """Column -> global query index for parity h (64-interleave)."""
    j = np.arange(1024)
    return 128 * (j // 64) + 64 * h + (j % 64)


def _host_prep(inputs_for_keys, inputs_for_values, inputs_for_queries,
               weight_q, weight_k, weight_v):
    f = lambda a: np.asarray(a, dtype=np.float32)
    ik, iv, iq = f(inputs_for_keys), f(inputs_for_values), f(inputs_for_queries)
    wq8 = _sbufize8(f(weight_q), WSCALE)
    wk8 = _sbufize8(f(weight_k), WSCALE)
    wv = _sbufize(f(weight_v))

    onesm = np.ones((P, P), np_bf16)
    p = np.arange(P, dtype=np.float32)
    thr = (128.0 * np.arange(16, dtype=np.float32))[None, :] + p[:, None]
    thr = np.ascontiguousarray(thr)          # thr[p, kt] = 128*kt + p
    in_maps = []
    for c in range(NCORES):
        b, h = c // 2, c % 2
        perm = _perm(h)
        xq = iq[b, perm]                      # [1024 cols, 1024 d]
        qglob = np.broadcast_to(perm.astype(np.float32), (P, 1024)).copy()
        in_maps.append({
            "wq8": wq8, "wk8": wk8, "wv": wv,
            "xq8": _sbufize8(np.ascontiguousarray(xq.T)),
            "xk8": _sbufize8(np.ascontiguousarray(ik[b].T)),
            "xvt": _sbufize(np.ascontiguousarray(iv[b].T)),
            "thr": thr, "qglob": qglob, "ones": onesm,
        })
    return in_maps


def _assemble(results):
    out = np.empty((B, S, D), np.float32)
    for c in range(NCORES):
        b, h = c // 2, c % 2
        oc = results[c]["out"].T        # [q_col, e]
        out[b, _perm(h)] = oc
    return out


def kernel(**inputs) -> np.ndarray:
    nc = _get_nc()
    in_maps = _host_prep(**inputs)
    res = run_bass_kernel_spmd(nc, in_maps, list(range(NCORES)))
    return _assemble(res.results)


def kernel_profiled(**inputs):
    """Like kernel() but also returns (output, exec_time_ns, results)."""
    nc = _get_nc()
    in_maps = _host_prep(**inputs)
    res = run_bass_kernel_spmd(nc, in_maps, list(range(NCORES)), trace=True)
    return _assemble(res.results), res.exec_time_ns, res
